# revision 1
# baseline (speedup 1.0000x reference)
"""Distributed GATv2 (2 layers + BN) Bass kernel for 8 trn2 NeuronCores.

Strategy: nodes partitioned by range across 8 cores (dst-ownership).
Each core:
  - computes BN1 stats partials -> AllReduce -> folds BN into Wl1/Wr1
  - computes xl1 = bn(x)@Wl1s for ALL nodes (bf16, local DRAM table)
  - computes xr1T (feat-major, + folded biases) for its own nodes
  - edge phase L1: per 128-edge subtile (degree-bucketed, dst-grouped):
      indirect-DMA gather of xl1[src] rows, feat-major z via PE
      (transpose-accumulate + identity-matmul of an AP-broadcast xr),
      LeakyReLU on ACT, logits via PE against block-diag att, exp with a
      global shift (softmax-invariant), transposed segment-sums numT/denT
      via one-hot matmuls, feat-major epilogue -> h1T (bf16)
  - one AllGather of h1T (+BN2 stat partials packed in 2 extra rows)
  - BN2 fold, xl2 table for all nodes, edge phase L2 (same scheme)
Output per core: outT [16, NODES_PAD] f32; host unpermutes/concats.
"""
import sys
import numpy as np

sys.path.insert(0, "/opt/trn_rl_repo")

import concourse.bass as bass          # noqa: E402
import concourse.bacc as bacc          # noqa: E402
import concourse.tile as tile          # noqa: E402
from concourse import mybir            # noqa: E402
from concourse.bass_utils import run_bass_kernel_spmd  # noqa: E402
from concourse.masks import make_identity  # noqa: E402

F32 = mybir.dt.float32
BF = mybir.dt.bfloat16
I32 = mybir.dt.int32
NPBF = mybir.dt.np(BF)

NCORES = 8
HEADS = 8
BN_EPS = 1e-5
NEG_SLOPE = 0.2
BUCKETS = (4, 8, 16, 32, 64, 128)


class Cfg:
    def __init__(self, n_nodes, in_dim, hid, out, m1, m2):
        self.N = n_nodes
        self.IN = in_dim
        self.HID = hid
        self.OUT = out
        self.F1 = HEADS * hid
        self.F2 = HEADS * out
        self.M1 = m1          # logit shift (softmax-invariant), layer 1
        self.M2 = m2
        self.NL = n_nodes // NCORES
        self.NT_PAD = ((n_nodes + 127) // 128) * 128
        self.KT = [min(128, in_dim), max(0, in_dim - 128)]  # K tiles for IN


def _schedule(cfg, deg_per_core):
    """Uniform-across-cores bucketed subtile schedule."""
    # counts per bucket per core
    t_b = {}
    for b in BUCKETS:
        lo = 0 if b == BUCKETS[0] else BUCKETS[BUCKETS.index(b) - 1]
        cnt = max(int(((d > lo) & (d <= b)).sum()) for d in deg_per_core)
        nps = 128 // b
        t_b[b] = ((cnt + nps - 1) // nps) * nps
    tot = sum(t_b.values())
    pad = (-tot) % 128
    # tot is a multiple of 4 (every t_b divisible by its nps>=1; smallest nps=1
    # for b=128 so not guaranteed -- fix by padding b=128 count to mult of 4)
    # simpler: bump b=BUCKETS[-1] (nps=1) then b=BUCKETS[0] (nps=32):
    b_last = BUCKETS[-1]
    add_last = pad % 32
    t_b[b_last] += add_last
    pad -= add_last
    t_b[BUCKETS[0]] += pad
    nodes_pad = sum(t_b.values())
    assert nodes_pad % 128 == 0
    subtiles = []   # (bucket, node_start)
    pos = 0
    for b in BUCKETS:
        nps = 128 // b
        assert t_b[b] % nps == 0
        for k in range(t_b[b] // nps):
            subtiles.append((b, pos))
            pos += nps
    assert pos == nodes_pad
    return t_b, nodes_pad, subtiles


def _preprocess(cfg, x, edge_index, W):
    N, NL = cfg.N, cfg.NL
    src = np.concatenate([edge_index[0], np.arange(N, dtype=np.int32)])
    dst = np.concatenate([edge_index[1], np.arange(N, dtype=np.int32)])
    order = np.argsort(dst, kind="stable")
    src, dst = src[order], dst[order]
    deg = np.bincount(dst, minlength=N)
    starts = np.zeros(N + 1, np.int64)
    np.cumsum(deg, out=starts[1:])
    deg_pc = [deg[c * NL:(c + 1) * NL] for c in range(NCORES)]
    t_b, NODES_PAD, subtiles = _schedule(cfg, deg_pc)
    NSUB = len(subtiles)
    NG = NODES_PAD // 128

    # per-core node processing order (bucket-sorted) and dummy slots (-1)
    proc = np.full((NCORES, NODES_PAD), -1, np.int64)   # proc pos -> local node
    ppos = np.full((NCORES, NL), -1, np.int64)          # local node -> proc pos
    for c in range(NCORES):
        pos = 0
        d = deg_pc[c]
        for b in BUCKETS:
            lo = 0 if b == BUCKETS[0] else BUCKETS[BUCKETS.index(b) - 1]
            ids = np.nonzero((d > lo) & (d <= b))[0]
            proc[c, pos:pos + len(ids)] = ids
            ppos[c, ids] = pos + np.arange(len(ids))
            pos += t_b[b]
    # storage index of a global node: core*NODES_PAD + ppos
    store = np.empty(N, np.int64)
    for c in range(NCORES):
        store[c * NL:(c + 1) * NL] = c * NODES_PAD + ppos[c]

    esrc1 = np.zeros((NCORES, 128, NSUB), np.int32)
    esrc2 = np.zeros((NCORES, 128, NSUB), np.int32)
    edsts = np.full((NCORES, 128, NSUB), -1.0, np.float32)
    for c in range(NCORES):
        for s, (b, nstart) in enumerate(subtiles):
            nps = 128 // b
            for slot in range(nps):
                v = proc[c, nstart + slot]
                if v < 0:
                    # dummy keep-alive edge so den > 0
                    edsts[c, slot * b, s] = slot
                    continue
                gv = c * NL + int(v)
                e0 = starts[gv]
                dv = int(deg[gv])
                p0 = slot * b
                esrc1[c, p0:p0 + dv, s] = src[e0:e0 + dv]
                esrc2[c, p0:p0 + dv, s] = store[src[e0:e0 + dv]]
                edsts[c, p0:p0 + dv, s] = slot

    # inputs per core
    xT = np.zeros((cfg.IN, cfg.NT_PAD), NPBF)
    xT[:, :N] = x.T.astype(NPBF)
    in_maps = []
    A1 = np.zeros((cfg.F1, HEADS), np.float32)
    for h in range(HEADS):
        A1[h * cfg.HID:(h + 1) * cfg.HID, h] = W["att1"][h]
    A2 = np.zeros((cfg.F2, HEADS), np.float32)
    for h in range(HEADS):
        A2[h * cfg.OUT:(h + 1) * cfg.OUT, h] = W["att2"][h]
    for c in range(NCORES):
        xTo = np.zeros((cfg.IN, NODES_PAD), NPBF)
        sel = proc[c] >= 0
        xTo[:, sel] = x[c * NL + proc[c][sel]].T.astype(NPBF)
        mask32 = np.zeros((cfg.HID, NODES_PAD), NPBF)
        mask32[:, sel] = 1.0
        mean1m = np.zeros((128, cfg.HID), np.float32)
        mean1m[np.arange(128), np.arange(128) % cfg.HID] = 0.125
        mean2m = np.zeros((128, cfg.OUT), np.float32)
        mean2m[np.arange(128), np.arange(128) % cfg.OUT] = 0.125
        e1m = np.zeros((8, cfg.F1), np.float32)
        e1m[np.arange(cfg.F1) // cfg.HID, np.arange(cfg.F1)] = 1.0
        e2m = np.zeros((8, cfg.F2), np.float32)
        e2m[np.arange(cfg.F2) // cfg.OUT, np.arange(cfg.F2)] = 1.0
        in_maps.append({
            "xT": xT, "xTo": xTo, "mask32": mask32,
            "mean1m": mean1m, "mean2m": mean2m, "e1m": e1m, "e2m": e2m,
            "esrc1": np.ascontiguousarray(esrc1[c]),
            "esrc2": np.ascontiguousarray(esrc2[c]),
            "edsts": np.ascontiguousarray(edsts[c]),
            "Wl1m": W["Wl1"].reshape(cfg.IN, HEADS, cfg.HID).mean(1).astype(np.float32),
            "Wl2m": W["Wl2"].reshape(cfg.HID, HEADS, cfg.OUT).mean(1).astype(np.float32),
            "Wl1": W["Wl1"].astype(np.float32),
            "Wr1": W["Wr1"].astype(np.float32),
            "Wl2": W["Wl2"].astype(np.float32),
            "Wr2": W["Wr2"].astype(np.float32),
            "A1": A1, "A2": A2,
            "gb1": np.stack([W["gamma1"], W["beta1"]], 1).astype(np.float32),
            "gb2": np.stack([W["gamma2"], W["beta2"]], 1).astype(np.float32),
            "b1c": W["b1"].reshape(-1, 1).astype(np.float32),
            "b2c": W["b2"].reshape(-1, 1).astype(np.float32),
        })
    meta = dict(NODES_PAD=NODES_PAD, NSUB=NSUB, NG=NG, subtiles=subtiles,
                proc=proc, in_maps=in_maps)
    return meta


def _build(cfg, meta):
    NODES_PAD, NSUB, NG = meta["NODES_PAD"], meta["NSUB"], meta["NG"]
    subtiles = meta["subtiles"]
    IN, F1, F2, HID, OUT = cfg.IN, cfg.F1, cfg.F2, cfg.HID, cfg.OUT
    K0, K1 = cfg.KT
    NTP = cfg.NT_PAD
    C1, C2 = HID, OUT
    n_xl1_tiles = NTP // 128
    n_xl2_tiles = NCORES * NODES_PAD // 128
    RECIP_N = 1.0 / cfg.N

    nc = bacc.Bacc("TRN2", target_bir_lowering=False, debug=False,
                   num_devices=NCORES)
    din = {}
    for name, shape, dt in [
            ("xT", [IN, NTP], BF), ("xTo", [IN, NODES_PAD], BF),
            ("mask32", [HID, NODES_PAD], BF),
            ("esrc1", [128, NSUB], I32), ("esrc2", [128, NSUB], I32),
            ("edsts", [128, NSUB], F32),
            ("Wl1", [IN, F1], F32), ("Wr1", [IN, F1], F32),
            ("Wl2", [HID, F2], F32), ("Wr2", [HID, F2], F32),
            ("A1", [F1, HEADS], F32), ("A2", [F2, HEADS], F32),
            ("gb1", [IN, 2], F32), ("gb2", [HID, 2], F32),
            ("b1c", [HID, 1], F32), ("b2c", [OUT, 1], F32),
            ("mean1m", [128, HID], F32), ("mean2m", [128, OUT], F32),
            ("Wl1m", [IN, HID], F32), ("Wl2m", [HID, OUT], F32),
            ("e1m", [8, F1], F32), ("e2m", [8, F2], F32)]:
        din[name] = nc.dram_tensor(name, shape, dt, kind="ExternalInput")
    outT = nc.dram_tensor("outT", [OUT, NODES_PAD], F32, kind="ExternalOutput")

    xl1_full = nc.dram_tensor("xl1_full", [NTP, F1], BF)
    xl2_full = nc.dram_tensor("xl2_full", [NCORES * NODES_PAD, F2], BF)
    st1_in = nc.dram_tensor("st1_in", [IN, 2], F32)
    bl1_d = nc.dram_tensor("bl1_d", [1, HID], F32)
    bl2_d = nc.dram_tensor("bl2_d", [1, OUT], F32)
    st1_out = nc.dram_tensor("st1_out", [IN, 2], F32)
    ag_in = nc.dram_tensor("ag_in", [HID + 2, NODES_PAD], BF)
    ag_out = nc.dram_tensor("ag_out", [NCORES * (HID + 2), NODES_PAD], BF,
                            addr_space="Shared")

    import contextlib
    with tile.TileContext(nc) as tc:
        ctx = contextlib.ExitStack()
        with ctx:
            cpool = ctx.enter_context(tc.tile_pool(name="const", bufs=1))
            rpool = ctx.enter_context(tc.tile_pool(name="resident", bufs=1))

            # ---------- constants ----------
            ident = cpool.tile([128, 128], BF)
            make_identity(nc, ident[:])
            ones_row = cpool.tile([1, 128], BF)
            nc.vector.memset(ones_row[:], 1.0)
            epsb = cpool.tile([128, 1], F32, tag="epsb")
            nc.vector.memset(epsb[:], BN_EPS)
            msh1 = cpool.tile([128, 1], F32, tag="msh1")
            nc.vector.memset(msh1[:], -float(cfg.M1))
            msh2 = cpool.tile([128, 1], F32, tag="msh2")
            nc.vector.memset(msh2[:], -float(cfg.M2))
            alph = cpool.tile([128, 1], F32, tag="alph")
            nc.vector.memset(alph[:], NEG_SLOPE)
            nslope = cpool.tile([128, 1], F32, tag="nslope")
            nc.vector.memset(nslope[:], -(1.0 - NEG_SLOPE))
            io_b = {}
            for b in set(bb for bb, _ in subtiles):
                nps = 128 // b
                it = cpool.tile([128, nps], I32, tag=f"io{b}i")
                nc.gpsimd.iota(it[:], [[1, nps]], channel_multiplier=0)
                ft = cpool.tile([128, nps], F32, tag=f"io{b}f")
                nc.vector.tensor_copy(out=ft[:], in_=it[:])
                io_b[b] = ft
            def const_bf(name, shape, tagn):
                tf = cpool.tile(shape, F32, tag=tagn + "f", name=tagn + "f")
                nc.sync.dma_start(out=tf[:], in_=din[name].ap())
                tb = cpool.tile(shape, BF, tag=tagn, name=tagn)
                nc.vector.tensor_copy(out=tb[:], in_=tf[:])
                return tb
            mean1 = const_bf("mean1m", [128, C1], "mean1")
            mean2 = const_bf("mean2m", [128, C2], "mean2")
            e1full = const_bf("e1m", [8, F1], "e1m")
            e2full = const_bf("e2m", [8, F2], "e2m")
            e1h = [e1full[:, h * 128:(h + 1) * 128] for h in range(F1 // 128)]
            e2h = [e2full[:, h * 128:(h + 1) * 128] for h in range(F2 // 128)]

            # index/dst tables resident
            esrc1_sb = rpool.tile([128, NSUB], I32)
            nc.sync.dma_start(out=esrc1_sb[:], in_=din["esrc1"].ap())
            esrc2_sb = rpool.tile([128, NSUB], I32)
            nc.sync.dma_start(out=esrc2_sb[:], in_=din["esrc2"].ap())
            edsts_sb = rpool.tile([128, NSUB], F32)
            nc.sync.dma_start(out=edsts_sb[:], in_=din["edsts"].ap())
            # A1/A2 bf16
            a1_sb = []
            for h in range(F1 // 128):
                t = rpool.tile([128, HEADS], BF, tag=f"a1_{h}")
                tf = rpool.tile([128, HEADS], F32, tag=f"a1f_{h}")
                nc.sync.dma_start(out=tf[:], in_=din["A1"].ap()[h * 128:(h + 1) * 128, :])
                nc.vector.tensor_copy(out=t[:], in_=tf[:])
                a1_sb.append(t)
            a2_sb = []
            for h in range(F2 // 128):
                t = rpool.tile([128, HEADS], BF, tag=f"a2_{h}")
                tf = rpool.tile([128, HEADS], F32, tag=f"a2f_{h}")
                nc.sync.dma_start(out=tf[:], in_=din["A2"].ap()[h * 128:(h + 1) * 128, :])
                nc.vector.tensor_copy(out=t[:], in_=tf[:])
                a2_sb.append(t)
            b1c_sb = rpool.tile([HID, 1], F32)
            nc.sync.dma_start(out=b1c_sb[:], in_=din["b1c"].ap())
            b2c_sb = rpool.tile([OUT, 1], F32)
            nc.sync.dma_start(out=b2c_sb[:], in_=din["b2c"].ap())
            mask32_sb = rpool.tile([HID, NODES_PAD], BF)
            nc.sync.dma_start(out=mask32_sb[:], in_=din["mask32"].ap())

            # xTo resident tiles (feat-major own nodes)
            kt_sizes = [K0] + ([K1] if K1 else [])
            xTo_sb = []
            for ki, ks in enumerate(kt_sizes):
                t = rpool.tile([ks, NODES_PAD], BF, tag=f"xTo{ki}")
                nc.sync.dma_start(out=t[:], in_=din["xTo"].ap()[ki * 128:ki * 128 + ks, :])
                xTo_sb.append(t)

            # ---------- phase A: BN1 stats + AllReduce ----------
            st_t = []
            with tc.tile_pool(name="pA", bufs=2) as pa, \
                 tc.tile_pool(name="pAs", bufs=1) as pas:
                for ki, ks in enumerate(kt_sizes):
                    st = pa.tile([ks, 2], F32, tag="st")
                    nc.vector.tensor_reduce(out=st[:, 0:1], in_=xTo_sb[ki][:],
                                            axis=mybir.AxisListType.X,
                                            op=mybir.AluOpType.add)
                    scr = pas.tile([ks, NODES_PAD], BF, tag="scr")
                    nc.scalar.activation(out=scr[:], in_=xTo_sb[ki][:],
                                         func=mybir.ActivationFunctionType.Square,
                                         accum_out=st[:, 1:2])
                    nc.sync.dma_start(out=st1_in.ap()[ki * 128:ki * 128 + ks, :],
                                      in_=st[:])
                    st_t.append(st)
            nc.gpsimd.collective_compute(
                "AllReduce", mybir.AluOpType.add,
                ins=[st1_in.ap()], outs=[st1_out.ap()],
                replica_groups=[list(range(NCORES))])

            # fold stats -> s1, t1 (per K tile) and scaled weights
            s1_t, t1_t = [], []
            wl1s, wr1s = [], []
            wl1f, wr1f = [], []
            with tc.tile_pool(name="pB", bufs=1) as pb:
                for ki, ks in enumerate(kt_sizes):
                    stg = pb.tile([ks, 2], F32, tag=f"stg{ki}")
                    nc.sync.dma_start(out=stg[:], in_=st1_out.ap()[ki * 128:ki * 128 + ks, :])
                    gb = pb.tile([ks, 2], F32, tag=f"gb{ki}")
                    nc.sync.dma_start(out=gb[:], in_=din["gb1"].ap()[ki * 128:ki * 128 + ks, :])
                    mean = pb.tile([ks, 1], F32, tag=f"mean{ki}")
                    nc.vector.tensor_scalar(out=mean[:], in0=stg[:, 0:1],
                                            scalar1=RECIP_N, scalar2=None,
                                            op0=mybir.AluOpType.mult)
                    q = pb.tile([ks, 1], F32, tag=f"q{ki}")
                    nc.vector.tensor_scalar(out=q[:], in0=stg[:, 1:2],
                                            scalar1=RECIP_N, scalar2=None,
                                            op0=mybir.AluOpType.mult)
                    m2 = pb.tile([ks, 1], F32, tag=f"m2{ki}")
                    nc.vector.tensor_tensor(out=m2[:], in0=mean[:], in1=mean[:],
                                            op=mybir.AluOpType.mult)
                    var = pb.tile([ks, 1], F32, tag=f"var{ki}")
                    nc.vector.tensor_tensor(out=var[:], in0=q[:], in1=m2[:],
                                            op=mybir.AluOpType.subtract)
                    sd = pb.tile([ks, 1], F32, tag=f"sd{ki}")
                    nc.scalar.activation(out=sd[:], in_=var[:],
                                         func=mybir.ActivationFunctionType.Sqrt,
                                         bias=epsb[:ks, :1])
                    rstd = pb.tile([ks, 1], F32, tag=f"rstd{ki}")
                    nc.vector.reciprocal(rstd[:], sd[:])
                    s1 = pb.tile([ks, 1], F32, tag=f"s1{ki}")
                    nc.vector.tensor_tensor(out=s1[:], in0=gb[:, 0:1], in1=rstd[:],
                                            op=mybir.AluOpType.mult)
                    ms = pb.tile([ks, 1], F32, tag=f"ms{ki}")
                    nc.vector.tensor_tensor(out=ms[:], in0=mean[:], in1=s1[:],
                                            op=mybir.AluOpType.mult)
                    t1 = pb.tile([ks, 1], F32, tag=f"t1{ki}")
                    nc.vector.tensor_tensor(out=t1[:], in0=gb[:, 1:2], in1=ms[:],
                                            op=mybir.AluOpType.subtract)
                    s1_t.append(s1)
                    t1_t.append(t1)
                    for wname, lst, lstf in (("Wl1", wl1s, wl1f), ("Wr1", wr1s, wr1f)):
                        wf = rpool.tile([ks, F1], F32, tag=f"{wname}f{ki}")
                        nc.sync.dma_start(out=wf[:], in_=din[wname].ap()[ki * 128:ki * 128 + ks, :])
                        ws = rpool.tile([ks, F1], BF, tag=f"{wname}s{ki}")
                        nc.vector.tensor_scalar(out=ws[:], in0=wf[:],
                                                scalar1=s1[:, :1], scalar2=None,
                                                op0=mybir.AluOpType.mult)
                        lst.append(ws)
                        lstf.append(wf)
                # bias12 = t1 @ (Wl1 + Wr1)  [1, F1]
                t1b = []
                for ki, ks in enumerate(kt_sizes):
                    tb = pb.tile([ks, 1], BF, tag=f"t1b{ki}")
                    nc.vector.tensor_copy(out=tb[:], in_=t1_t[ki][:])
                    t1b.append(tb)
                with tc.tile_pool(name="pBp", bufs=1, space="PSUM") as pbp:
                    bps = pbp.tile([1, F1], F32, space="PSUM")
                    first = True
                    for ki, ks in enumerate(kt_sizes):
                        for wf in (wl1f[ki], wr1f[ki]):
                            wb = pb.tile([ks, F1], BF, tag=f"wb{ki}")
                            nc.vector.tensor_copy(out=wb[:], in_=wf[:])
                            nc.tensor.matmul(out=bps[:], lhsT=t1b[ki][:], rhs=wb[:],
                                             start=first, stop=(ki == len(kt_sizes) - 1 and wf is wr1f[ki]))
                            first = False
                    bias12 = rpool.tile([1, F1], BF)
                    nc.vector.tensor_copy(out=bias12[:], in_=bps[:])
                with tc.tile_pool(name="pBq", bufs=1, space="PSUM") as pbq:
                    blp = pbq.tile([1, HID], F32, space="PSUM")
                    for ki, ks in enumerate(kt_sizes):
                        wmf = pb.tile([ks, HID], F32, tag=f"wmf{ki}",
                                      name=f"wmf{ki}")
                        nc.sync.dma_start(
                            out=wmf[:],
                            in_=din["Wl1m"].ap()[ki * 128:ki * 128 + ks, :])
                        wmb = pb.tile([ks, HID], BF, tag=f"wmb{ki}",
                                      name=f"wmb{ki}")
                        nc.vector.tensor_copy(out=wmb[:], in_=wmf[:])
                        nc.tensor.matmul(out=blp[:], lhsT=t1b[ki][:], rhs=wmb[:],
                                         start=(ki == 0),
                                         stop=(ki == len(kt_sizes) - 1))
                    blr = pb.tile([1, HID], F32, tag="blr")
                    nc.vector.tensor_copy(out=blr[:], in_=blp[:])
                    nc.sync.dma_start(out=bl1_d.ap(), in_=blr[:])
                blc = rpool.tile([HID, 1], F32)
                nc.sync.dma_start(out=blc[:], in_=bl1_d.ap())
                bias1t = rpool.tile([HID, 1], F32)
                nc.vector.tensor_tensor(out=bias1t[:], in0=blc[:], in1=b1c_sb[:],
                                        op=mybir.AluOpType.add)

            # ---------- phase A2: xr1T resident (feat-major, own nodes) ----------
            xr1T = []
            with tc.tile_pool(name="pC", bufs=2, space="PSUM") as pc, \
                 tc.tile_pool(name="pCs", bufs=2) as pcs:
                for g in range(NG):
                    halves = []
                    for h in range(F1 // 128):
                        ps = pc.tile([128, 128], F32, space="PSUM", tag="xr1p")
                        for ki, ks in enumerate(kt_sizes):
                            nc.tensor.matmul(
                                out=ps[:],
                                lhsT=wr1s[ki][:, h * 128:(h + 1) * 128],
                                rhs=xTo_sb[ki][:, g * 128:(g + 1) * 128],
                                start=(ki == 0), stop=False)
                        nc.tensor.matmul(out=ps[:],
                                         lhsT=bias12[:, h * 128:(h + 1) * 128],
                                         rhs=ones_row[:],
                                         start=False, stop=True)
                        t = rpool.tile([128, 128], BF, tag=f"xr1T_{g}_{h}")
                        eng = nc.vector if (g + h) % 2 == 0 else nc.scalar
                        if eng is nc.vector:
                            nc.vector.tensor_copy(out=t[:], in_=ps[:])
                        else:
                            nc.scalar.copy(out=t[:], in_=ps[:])
                        halves.append(t)
                    xr1T.append(halves)

            # ---------- phase A3: xl1_full table ----------
            with tc.tile_pool(name="pD", bufs=3) as pd, \
                 tc.tile_pool(name="pDp", bufs=2, space="PSUM") as pdp:
                for t in range(n_xl1_tiles):
                    lhs = []
                    for ki, ks in enumerate(kt_sizes):
                        lt = pd.tile([ks, 128], BF, tag=f"xl1l{ki}")
                        nc.sync.dma_start(
                            out=lt[:],
                            in_=din["xT"].ap()[ki * 128:ki * 128 + ks,
                                               t * 128:(t + 1) * 128])
                        lhs.append(lt)
                    ps = pdp.tile([128, F1], F32, space="PSUM", tag="xl1p")
                    for ki, ks in enumerate(kt_sizes):
                        nc.tensor.matmul(out=ps[:], lhsT=lhs[ki][:], rhs=wl1s[ki][:],
                                         start=(ki == 0), stop=(ki == len(kt_sizes) - 1))
                    ob = pd.tile([128, F1], BF, tag="xl1o")
                    if t % 2 == 0:
                        nc.vector.tensor_copy(out=ob[:], in_=ps[:])
                    else:
                        nc.scalar.copy(out=ob[:], in_=ps[:])
                    nc.sync.dma_start(out=xl1_full.ap()[t * 128:(t + 1) * 128, :],
                                      in_=ob[:])

            # ---------- helper: edge phase ----------
            def edge_phase(F, xfull, esrc_sb, ah_sb, eh_mats, meanm, shift_ap,
                           bias_col, out_cb, layer):
                nhalf = F // 128
                groups = {}
                for s, (b, nstart) in enumerate(subtiles):
                    groups.setdefault(nstart // 128, []).append((s, b, nstart % 128))
                with tc.tile_pool(name=f"ge{layer}", bufs=6) as gp, \
                     tc.tile_pool(name=f"gz{layer}", bufs=2, space="PSUM") as gz, \
                     tc.tile_pool(name=f"gl{layer}", bufs=1, space="PSUM") as gl, \
                     tc.tile_pool(name=f"gn{layer}", bufs=2, space="PSUM") as gn, \
                     tc.tile_pool(name=f"gd{layer}", bufs=1, space="PSUM") as gd, \
                     tc.tile_pool(name=f"gs{layer}", bufs=4) as gs:
                    for g in range(NG):
                        subs = groups[g]
                        numT = gn.tile([128, nhalf * 128], F32, space="PSUM",
                                       tag="numT")
                        denT = gd.tile([8, 128], F32, space="PSUM", tag="denT")
                        # batches of 4 subtiles
                        for b0 in range(0, len(subs), 4):
                            batch = subs[b0:b0 + 4]
                            nb = len(batch)
                            zts = []
                            for h in range(nhalf):
                                zth = gz.tile([128, nb * 128], F32, space="PSUM",
                                              tag=f"zt{h}", name=f"zt{h}")
                                zts.append(zth)
                            lg = gl.tile([128, nb * 8], F32, space="PSUM", tag="lg")
                            xls_b = []
                            for si, (s, b, noff) in enumerate(batch):
                                xls = gp.tile([128, F], BF, tag="xls")
                                nc.gpsimd.indirect_dma_start(
                                    out=xls[:], out_offset=None,
                                    in_=xfull.ap(),
                                    in_offset=bass.IndirectOffsetOnAxis(
                                        ap=esrc_sb[:, s:s + 1], axis=0))
                                xls_b.append(xls)
                                nps = 128 // b
                                for h in range(nhalf):
                                    zsl = zts[h][:, si * 128:(si + 1) * 128]
                                    nc.tensor.matmul(
                                        out=zsl, lhsT=xls[:, h * 128:(h + 1) * 128],
                                        rhs=ident[:],
                                        start=True, stop=False)
                                    xr_ap = (xr1T[g][h] if layer == 1 else xr2T[g])
                                    rep = xr_ap[:, noff:noff + nps, None] \
                                        .broadcast_to([128, nps, b])
                                    nc.tensor.matmul(out=zsl, lhsT=ident[:],
                                                     rhs=rep, start=False,
                                                     stop=True)
                            es = []
                            for h in range(nhalf):
                                # lrelu(z) = z + relu(-0.8 z)
                                r8 = gs.tile([128, nb * 128], BF, tag=f"r8{h}",
                                             name=f"r8{h}")
                                nc.scalar.activation(
                                    out=r8[:], in_=zts[h][:],
                                    func=mybir.ActivationFunctionType.Relu,
                                    scale=nslope[:, :1])
                                e_sb = gs.tile([128, nb * 128], BF, tag=f"es{h}",
                                               name=f"es{h}")
                                nc.vector.tensor_tensor(
                                    out=e_sb[:], in0=zts[h][:], in1=r8[:],
                                    op=mybir.AluOpType.add)
                                es.append(e_sb)
                            for si, (s, b, noff) in enumerate(batch):
                                for h in range(nhalf):
                                    nc.tensor.matmul(
                                        out=lg[:, si * 8:(si + 1) * 8],
                                        lhsT=es[h][:, si * 128:(si + 1) * 128],
                                        rhs=ah_sb[h][:],
                                        start=(h == 0), stop=(h == nhalf - 1))
                            w4 = gs.tile([128, nb * 8], BF, tag="w4")
                            nc.scalar.activation(
                                out=w4[:], in_=lg[:],
                                func=mybir.ActivationFunctionType.Exp,
                                bias=shift_ap[:, :1])
                            for si, (s, b, noff) in enumerate(batch):
                                nps = 128 // b
                                S_t = gs.tile([128, nps], BF, tag="S")
                                nc.vector.tensor_scalar(
                                    out=S_t[:], in0=io_b[b][:],
                                    scalar1=edsts_sb[:, s:s + 1], scalar2=None,
                                    op0=mybir.AluOpType.is_equal)
                                y = gp.tile([128, F], BF, tag="y")
                                wv = w4[:, si * 8:(si + 1) * 8, None] \
                                    .broadcast_to([128, 8, F // 8])
                                nc.vector.tensor_tensor(
                                    out=y[:].rearrange("p (a b) -> p a b", a=8),
                                    in0=xls_b[si][:].rearrange(
                                        "p (a b) -> p a b", a=8),
                                    in1=wv, op=mybir.AluOpType.mult)
                                for h in range(nhalf):
                                    nc.tensor.matmul(
                                        out=numT[:, h * 128 + noff:h * 128 + noff + nps],
                                        lhsT=y[:, h * 128:(h + 1) * 128],
                                        rhs=S_t[:], start=True, stop=True)
                                nc.tensor.matmul(
                                    out=denT[0:8, noff:noff + nps],
                                    lhsT=w4[:, si * 8:(si + 1) * 8],
                                    rhs=S_t[:], start=True, stop=True)
                        # ---- group epilogue ----
                        drec = gs.tile([8, 128], F32, tag="drec")
                        nc.vector.reciprocal(drec[:], denT[:])
                        drecb = gs.tile([8, 128], BF, tag="drecb")
                        nc.vector.tensor_copy(out=drecb[:], in_=drec[:])
                        onts = []
                        for h in range(nhalf):
                            rexp = gz.tile([128, 128], F32, space="PSUM",
                                           tag=f"zt{h}")
                            nc.tensor.matmul(out=rexp[:], lhsT=eh_mats[h],
                                             rhs=drecb[:], start=True, stop=True)
                            rexpb = gs.tile([128, 128], BF, tag=f"rexpb{h}",
                                            name=f"rexpb{h}")
                            nc.scalar.copy(out=rexpb[:], in_=rexp[:])
                            ont = gs.tile([128, 128], BF, tag=f"ont{h}",
                                          name=f"ont{h}")
                            nc.vector.tensor_tensor(
                                out=ont[:], in0=numT[:, h * 128:(h + 1) * 128],
                                in1=rexpb[:], op=mybir.AluOpType.mult)
                            onts.append(ont)
                        cdim = C1 if layer == 1 else C2
                        ot = gl.tile([cdim, 128], F32, space="PSUM", tag="lg")
                        for h in range(nhalf):
                            nc.tensor.matmul(out=ot[:], lhsT=meanm[:, :cdim],
                                             rhs=onts[h][:], start=(h == 0),
                                             stop=(h == nhalf - 1))
                        out_cb(g, ot, bias_col)

            # ---------- phase B: layer-1 edges -> h1T ----------
            h1T = rpool.tile([HID, NODES_PAD], BF)
            oB = ctx.enter_context(tc.tile_pool(name="oB", bufs=2))

            def l1_out(g, ot_psum, bias_col):
                hrel = oB.tile([HID, 128], BF, tag="hrel")
                nc.scalar.activation(out=hrel[:], in_=ot_psum[:],
                                     func=mybir.ActivationFunctionType.Relu,
                                     bias=bias_col[:, :1])
                nc.vector.tensor_tensor(out=h1T[:, g * 128:(g + 1) * 128],
                                        in0=hrel[:],
                                        in1=mask32_sb[:, g * 128:(g + 1) * 128],
                                        op=mybir.AluOpType.mult)

            edge_phase(F1, xl1_full, esrc1_sb, a1_sb, e1h, mean1, msh1,
                       bias1t, l1_out, layer=1)

            # ---------- phase C: AllGather h1T + BN2 + xl2 + xr2T ----------
            with tc.tile_pool(name="pE", bufs=2) as pe:
                st2 = pe.tile([HID, 2], F32, tag="st2")
                nc.vector.tensor_reduce(out=st2[:, 0:1], in_=h1T[:],
                                        axis=mybir.AxisListType.X,
                                        op=mybir.AluOpType.add)
                scr2 = pe.tile([HID, NODES_PAD], BF, tag="scr2")
                nc.scalar.activation(out=scr2[:], in_=h1T[:],
                                     func=mybir.ActivationFunctionType.Square,
                                     accum_out=st2[:, 1:2])
                nc.sync.dma_start(out=ag_in.ap()[0:HID, :], in_=h1T[:])
                # stats rows (f32 bitcast into bf16 row space)
                nc.sync.dma_start(out=ag_in.ap()[HID:HID + 1, 0:2 * HID],
                                  in_=st2[:, 0:1].bitcast(BF))
                nc.sync.dma_start(out=ag_in.ap()[HID + 1:HID + 2, 0:2 * HID],
                                  in_=st2[:, 1:2].bitcast(BF))
            nc.gpsimd.collective_compute(
                "AllGather", mybir.AluOpType.bypass,
                ins=[ag_in.ap()], outs=[ag_out.ap()],
                replica_groups=[list(range(NCORES))])

            with tc.tile_pool(name="pF", bufs=1) as pf:
                # gather stat rows: each row HID f32 (=2*HID bf16)
                s2sum = pf.tile([HID, NCORES], F32, tag="s2sum")
                s2sq = pf.tile([HID, NCORES], F32, tag="s2sq")
                agf = ag_out.ap().bitcast(F32)  # [272, NODES_PAD//2]
                for c in range(NCORES):
                    r = c * (HID + 2) + HID
                    nc.sync.dma_start(out=s2sum[:, c:c + 1],
                                      in_=agf[r:r + 1, 0:HID])
                    nc.sync.dma_start(out=s2sq[:, c:c + 1],
                                      in_=agf[r + 1:r + 2, 0:HID])
                stg = pf.tile([HID, 2], F32, tag="stg2")
                nc.vector.tensor_reduce(out=stg[:, 0:1], in_=s2sum[:],
                                        axis=mybir.AxisListType.X,
                                        op=mybir.AluOpType.add)
                nc.vector.tensor_reduce(out=stg[:, 1:2], in_=s2sq[:],
                                        axis=mybir.AxisListType.X,
                                        op=mybir.AluOpType.add)
                gb = pf.tile([HID, 2], F32, tag="gb2")
                nc.sync.dma_start(out=gb[:], in_=din["gb2"].ap())
                mean = pf.tile([HID, 1], F32, tag="mean2")
                nc.vector.tensor_scalar(out=mean[:], in0=stg[:, 0:1],
                                        scalar1=RECIP_N, scalar2=None,
                                        op0=mybir.AluOpType.mult)
                q = pf.tile([HID, 1], F32, tag="q2")
                nc.vector.tensor_scalar(out=q[:], in0=stg[:, 1:2],
                                        scalar1=RECIP_N, scalar2=None,
                                        op0=mybir.AluOpType.mult)
                m2 = pf.tile([HID, 1], F32, tag="m22")
                nc.vector.tensor_tensor(out=m2[:], in0=mean[:], in1=mean[:],
                                        op=mybir.AluOpType.mult)
                var = pf.tile([HID, 1], F32, tag="var2")
                nc.vector.tensor_tensor(out=var[:], in0=q[:], in1=m2[:],
                                        op=mybir.AluOpType.subtract)
                sd = pf.tile([HID, 1], F32, tag="sd2")
                nc.scalar.activation(out=sd[:], in_=var[:],
                                     func=mybir.ActivationFunctionType.Sqrt,
                                     bias=epsb[:HID, :1])
                rstd = pf.tile([HID, 1], F32, tag="rstd2")
                nc.vector.reciprocal(rstd[:], sd[:])
                s2 = pf.tile([HID, 1], F32, tag="s2")
                nc.vector.tensor_tensor(out=s2[:], in0=gb[:, 0:1], in1=rstd[:],
                                        op=mybir.AluOpType.mult)
                ms = pf.tile([HID, 1], F32, tag="ms2")
                nc.vector.tensor_tensor(out=ms[:], in0=mean[:], in1=s2[:],
                                        op=mybir.AluOpType.mult)
                t2 = pf.tile([HID, 1], F32, tag="t2")
                nc.vector.tensor_tensor(out=t2[:], in0=gb[:, 1:2], in1=ms[:],
                                        op=mybir.AluOpType.subtract)
                # scaled weights
                wl2f = pf.tile([HID, F2], F32, tag="wl2f")
                nc.sync.dma_start(out=wl2f[:], in_=din["Wl2"].ap())
                wr2f = pf.tile([HID, F2], F32, tag="wr2f")
                nc.sync.dma_start(out=wr2f[:], in_=din["Wr2"].ap())
                wl2s = rpool.tile([HID, F2], BF)
                nc.vector.tensor_scalar(out=wl2s[:], in0=wl2f[:],
                                        scalar1=s2[:, :1], scalar2=None,
                                        op0=mybir.AluOpType.mult)
                wr2s = rpool.tile([HID, F2], BF)
                nc.vector.tensor_scalar(out=wr2s[:], in0=wr2f[:],
                                        scalar1=s2[:, :1], scalar2=None,
                                        op0=mybir.AluOpType.mult)
                t2b = pf.tile([HID, 1], BF, tag="t2b")
                nc.vector.tensor_copy(out=t2b[:], in_=t2[:])
                wsum = pf.tile([HID, F2], BF, tag="wsum")
                nc.vector.tensor_tensor(out=wsum[:], in0=wl2f[:], in1=wr2f[:],
                                        op=mybir.AluOpType.add)
                with tc.tile_pool(name="pFp", bufs=1, space="PSUM") as pfp:
                    bps = pfp.tile([1, F2], F32, space="PSUM")
                    nc.tensor.matmul(out=bps[:], lhsT=t2b[:], rhs=wsum[:],
                                     start=True, stop=True)
                    bias22 = rpool.tile([1, F2], BF)
                    nc.vector.tensor_copy(out=bias22[:], in_=bps[:])
                with tc.tile_pool(name="pFq", bufs=1, space="PSUM") as pfq:
                    wmf2 = pf.tile([HID, OUT], F32, tag="wmf2")
                    nc.sync.dma_start(out=wmf2[:], in_=din["Wl2m"].ap())
                    wmb2 = pf.tile([HID, OUT], BF, tag="wmb2")
                    nc.vector.tensor_copy(out=wmb2[:], in_=wmf2[:])
                    blp2 = pfq.tile([1, OUT], F32, space="PSUM")
                    nc.tensor.matmul(out=blp2[:], lhsT=t2b[:], rhs=wmb2[:],
                                     start=True, stop=True)
                    blr2 = pf.tile([1, OUT], F32, tag="blr2")
                    nc.vector.tensor_copy(out=blr2[:], in_=blp2[:])
                    nc.sync.dma_start(out=bl2_d.ap(), in_=blr2[:])
                blc2 = rpool.tile([OUT, 1], F32)
                nc.sync.dma_start(out=blc2[:], in_=bl2_d.ap())
                bias2t = rpool.tile([OUT, 1], F32)
                nc.vector.tensor_tensor(out=bias2t[:], in0=blc2[:], in1=b2c_sb[:],
                                        op=mybir.AluOpType.add)

            # xl2_full
            with tc.tile_pool(name="pG", bufs=3) as pg, \
                 tc.tile_pool(name="pGp", bufs=2, space="PSUM") as pgp:
                for t in range(n_xl2_tiles):
                    c_src = (t * 128) // NODES_PAD
                    off = (t * 128) % NODES_PAD
                    lhs2 = pg.tile([HID, 128], BF, tag="xl2l")
                    nc.sync.dma_start(
                        out=lhs2[:],
                        in_=ag_out.ap()[c_src * (HID + 2):c_src * (HID + 2) + HID,
                                        off:off + 128])
                    ps = pgp.tile([128, F2], F32, space="PSUM", tag="xl2p")
                    nc.tensor.matmul(out=ps[:], lhsT=lhs2[:],
                                     rhs=wl2s[:], start=True, stop=True)
                    ob = pg.tile([128, F2], BF, tag="xl2o")
                    if t % 2 == 0:
                        nc.vector.tensor_copy(out=ob[:], in_=ps[:])
                    else:
                        nc.scalar.copy(out=ob[:], in_=ps[:])
                    nc.sync.dma_start(out=xl2_full.ap()[t * 128:(t + 1) * 128, :],
                                      in_=ob[:])
            # xr2T resident
            xr2T = []
            with tc.tile_pool(name="pH", bufs=2, space="PSUM") as ph:
                for g in range(NG):
                    ps = ph.tile([128, 128], F32, space="PSUM", tag="xr2p")
                    nc.tensor.matmul(out=ps[:], lhsT=wr2s[:],
                                     rhs=h1T[:, g * 128:(g + 1) * 128],
                                     start=True, stop=False)
                    nc.tensor.matmul(out=ps[:], lhsT=bias22[:], rhs=ones_row[:],
                                     start=False, stop=True)
                    t = rpool.tile([128, 128], BF, tag=f"xr2T_{g}")
                    if g % 2 == 0:
                        nc.vector.tensor_copy(out=t[:], in_=ps[:])
                    else:
                        nc.scalar.copy(out=t[:], in_=ps[:])
                    xr2T.append(t)

            # ---------- phase D: layer-2 edges -> outT ----------
            oD = ctx.enter_context(tc.tile_pool(name="oD", bufs=2))

            def l2_out(g, ot_psum, bias_col):
                ob = oD.tile([OUT, 128], F32, tag="ob")
                nc.scalar.activation(out=ob[:], in_=ot_psum[:],
                                     func=mybir.ActivationFunctionType.Identity,
                                     bias=bias_col[:, :1])
                nc.sync.dma_start(out=outT.ap()[:, g * 128:(g + 1) * 128],
                                  in_=ob[:])

            edge_phase(F2, xl2_full, esrc2_sb, a2_sb, e2h, mean2, msh2,
                       bias2t, l2_out, layer=2)

    nc.compile()
    return nc


_CACHE = {}


def _get_nc(cfg, meta):
    key = (cfg.N, cfg.IN, cfg.HID, cfg.OUT, meta["NSUB"], meta["NODES_PAD"])
    if key not in _CACHE:
        _CACHE[key] = _build(cfg, meta)
    return _CACHE[key]


def run(cfg, inputs):
    x = np.asarray(inputs["x"], np.float32)
    ei = np.asarray(inputs["edge_index"], np.int32)
    W = {k: np.asarray(inputs[k], np.float32) for k in
         ("Wl1", "Wr1", "att1", "b1", "gamma1", "beta1",
          "Wl2", "Wr2", "att2", "b2", "gamma2", "beta2")}
    meta = _preprocess(cfg, x, ei, W)
    nc = _get_nc(cfg, meta)
    res = run_bass_kernel_spmd(nc, meta["in_maps"], core_ids=list(range(NCORES)))
    out = np.empty((cfg.N, cfg.OUT), np.float32)
    proc = meta["proc"]
    for c in range(NCORES):
        oT = res.results[c]["outT"]      # [OUT, NODES_PAD]
        sel = proc[c] >= 0
        out[c * cfg.NL + proc[c][sel]] = oT[:, sel].T
    return out, meta, nc


def kernel(**inputs):
    cfg = Cfg(50000, 200, 32, 16, m1=8.0, m2=10.0)
    out, _, _ = run(cfg, inputs)
    return out



# revision 16
# speedup vs baseline: 15.3717x; 15.3717x over previous
"""Distributed GATv2 (2 layers + BN) Bass kernel for 8 trn2 NeuronCores. v2

Strategy: nodes dealt round-robin by degree across 8 cores (dst-ownership,
degree-balanced so the SPMD subtile schedule is identical on every core).
Each core:
  - BN1 stats partials on own nodes -> AllReduce -> s1/t1 applied via
    Act-engine scale/bias on x tiles (no weight folding)
  - xl1 table = bn(x)@Wl1 for ALL nodes (bf16 DRAM, batched DMA)
  - xr1T = Wr1^T @ bn(x_own) feat-major resident
  - edge phase L1: subtiles of 128 edge slots, exact-degree packing
    (nps nodes x d edges, tail slots dead). Batched indirect gather of
    xl1[src] (16 subtiles/DMA); per subtile: PE transpose + xr broadcast
    add -> z (PSUM), one DVE scalar_tensor_tensor LeakyReLU, PE logits vs
    block-diag att, Act exp, DVE alpha-weighting, PE one-hot
    (host-precomputed S_all) segment sums numT/denT, group epilogue with
    clamped-denominator reciprocal -> h1T
  - AllGather h1T (+BN2 stats rows) -> BN2 via Act scale/bias -> xl2
    table + xr2T -> edge phase L2 -> outT (single store)
Output per core: outT [16, NODES_PAD] f32; host unpermutes via proc map.
"""
import sys
import numpy as np

sys.path.insert(0, "/opt/trn_rl_repo")

import concourse.bass as bass          # noqa: E402
import concourse.bacc as bacc          # noqa: E402
import concourse.tile as tile          # noqa: E402
from concourse import mybir            # noqa: E402
from concourse.bass_utils import run_bass_kernel_spmd  # noqa: E402
from concourse.masks import make_identity  # noqa: E402

F32 = mybir.dt.float32
BF = mybir.dt.bfloat16
I32 = mybir.dt.int32
NPBF = mybir.dt.np(BF)

NCORES = 8
HEADS = 8
BN_EPS = 1e-5
NEG_SLOPE = 0.2


class Cfg:
    def __init__(self, n_nodes, in_dim, hid, out, m1, m2):
        self.N = n_nodes
        self.IN = in_dim
        self.HID = hid
        self.OUT = out
        self.F1 = HEADS * hid
        self.F2 = HEADS * out
        self.M1 = m1          # logit shift (softmax-invariant), layer 1
        self.M2 = m2
        self.NT_PAD = ((n_nodes + 127) // 128) * 128
        self.KT = [min(128, in_dim), max(0, in_dim - 128)]  # K tiles for IN


def _schedule_v2(dmax):
    """Exact-degree subtile schedule from cross-core max per-degree counts.
    Returns subtiles [(d, take, nstart)], NODES_PAD, NSUB."""
    subtiles = []
    pos = 0
    for d in sorted(dmax):
        cnt = dmax[d]
        nps = 128 // d
        while cnt > 0:
            take = min(nps, cnt)
            cnt -= take
            if pos // 128 != (pos + take - 1) // 128:
                pos = ((pos // 128) + 1) * 128
            subtiles.append((d, take, pos))
            pos += take
    nodes_pad = ((pos + 127) // 128) * 128
    return subtiles, nodes_pad, len(subtiles)


def _preprocess(cfg, x, edge_index, W):
    N = cfg.N
    src = np.concatenate([edge_index[0], np.arange(N, dtype=np.int32)])
    dst = np.concatenate([edge_index[1], np.arange(N, dtype=np.int32)])
    order = np.argsort(dst, kind="stable")
    src, dst = src[order], dst[order]
    deg = np.bincount(dst, minlength=N)
    starts = np.zeros(N + 1, np.int64)
    np.cumsum(deg, out=starts[1:])

    # degree-balanced node->core assignment (round-robin by degree rank)
    nodes_by_deg = np.lexsort((np.arange(N), deg))
    node_core = np.empty(N, np.int64)
    node_core[nodes_by_deg] = np.arange(N) % NCORES
    dmax = {}
    own_by_core = []
    for c in range(NCORES):
        own = nodes_by_deg[node_core[nodes_by_deg] == c]
        own_by_core.append(own)
        dd, cc = np.unique(deg[own], return_counts=True)
        for d, k in zip(dd, cc):
            dmax[int(d)] = max(dmax.get(int(d), 0), int(k))
    subtiles, NODES_PAD, NSUB = _schedule_v2(dmax)
    NG = NODES_PAD // 128

    proc = np.full((NCORES, NODES_PAD), -1, np.int64)  # slot -> global node
    for c in range(NCORES):
        by_d = {}
        for v in own_by_core[c]:
            by_d.setdefault(int(deg[v]), []).append(int(v))
        for d, take, nstart in subtiles:
            lst = by_d.get(d, [])
            t = min(take, len(lst))
            for j in range(t):
                proc[c, nstart + j] = lst[j]
            by_d[d] = lst[t:]
        assert all(len(v) == 0 for v in by_d.values())
    store = np.full(N, -1, np.int64)
    for c in range(NCORES):
        sel = proc[c] >= 0
        store[proc[c][sel]] = c * NODES_PAD + np.nonzero(sel)[0]
    assert (store >= 0).all()

    ZROW1 = cfg.NT_PAD - 1          # reserved zero row in xl1_full
    esrc1 = np.full((NCORES, 128, NSUB), ZROW1, np.int32)
    esrc2 = np.full((NCORES, 128, NSUB), NCORES * NODES_PAD, np.int32)
    S_all = np.zeros((128, NODES_PAD), NPBF)
    for s, (d, take, nstart) in enumerate(subtiles):
        for j in range(take):
            S_all[j * d:(j + 1) * d, nstart + j] = 1.0
    for c in range(NCORES):
        for s, (d, take, nstart) in enumerate(subtiles):
            for j in range(take):
                v = proc[c, nstart + j]
                if v < 0:
                    continue
                e0 = starts[v]
                dv = int(deg[v])
                assert dv == d
                p0 = j * d
                esrc1[c, p0:p0 + dv, s] = src[e0:e0 + dv]
                esrc2[c, p0:p0 + dv, s] = store[src[e0:e0 + dv]]

    xT = np.zeros((cfg.IN, cfg.NT_PAD), NPBF)
    xT[:, :N] = x.T.astype(NPBF)
    A1 = np.zeros((cfg.F1, HEADS), np.float32)
    for h in range(HEADS):
        A1[h * cfg.HID:(h + 1) * cfg.HID, h] = W["att1"][h]
    A2 = np.zeros((cfg.F2, HEADS), np.float32)
    for h in range(HEADS):
        A2[h * cfg.OUT:(h + 1) * cfg.OUT, h] = W["att2"][h]
    mean1m = np.zeros((128, cfg.HID), np.float32)
    mean1m[np.arange(128), np.arange(128) % cfg.HID] = 0.125
    mean2m = np.zeros((128, cfg.OUT), np.float32)
    mean2m[np.arange(128), np.arange(128) % cfg.OUT] = 0.125
    e1m = np.zeros((8, cfg.F1), np.float32)
    e1m[np.arange(cfg.F1) // cfg.HID, np.arange(cfg.F1)] = 1.0
    e2m = np.zeros((8, cfg.F2), np.float32)
    e2m[np.arange(cfg.F2) // cfg.OUT, np.arange(cfg.F2)] = 1.0

    in_maps = []
    for c in range(NCORES):
        xTo = np.zeros((cfg.IN, NODES_PAD), NPBF)
        sel = proc[c] >= 0
        xTo[:, sel] = x[proc[c][sel]].T.astype(NPBF)
        ndead = NODES_PAD - int(sel.sum())
        v = np.maximum(W["b1"], 0.0).astype(NPBF).astype(np.float32)
        dstat = np.stack([ndead * v, ndead * v * v], 1).astype(np.float32)
        in_maps.append({
            "xT": xT, "xTo": xTo, "dstat": dstat,
            "S_all": S_all,
            "mean1m": mean1m.astype(NPBF), "mean2m": mean2m.astype(NPBF),
            "e1m": e1m.astype(NPBF), "e2m": e2m.astype(NPBF),
            "esrc1": np.ascontiguousarray(esrc1[c]),
            "esrc2": np.ascontiguousarray(esrc2[c]),
            "Wl1": W["Wl1"].astype(NPBF), "Wr1": W["Wr1"].astype(NPBF),
            "Wl2": W["Wl2"].astype(NPBF), "Wr2": W["Wr2"].astype(NPBF),
            "A1": A1.astype(NPBF), "A2": A2.astype(NPBF),
            "gb1": np.stack([W["gamma1"], W["beta1"]], 1).astype(np.float32),
            "gb2": np.stack([W["gamma2"], W["beta2"]], 1).astype(np.float32),
            "b1c": W["b1"].reshape(-1, 1).astype(np.float32),
            "b2c": W["b2"].reshape(-1, 1).astype(np.float32),
        })
    meta = dict(NODES_PAD=NODES_PAD, NSUB=NSUB, NG=NG, subtiles=subtiles,
                proc=proc, in_maps=in_maps)
    return meta


def _build(cfg, meta):
    NODES_PAD, NSUB, NG = meta["NODES_PAD"], meta["NSUB"], meta["NG"]
    subtiles = meta["subtiles"]
    IN, F1, F2, HID, OUT = cfg.IN, cfg.F1, cfg.F2, cfg.HID, cfg.OUT
    NTP = cfg.NT_PAD
    RECIP_N = 1.0 / cfg.N
    K0, K1 = cfg.KT
    kt_sizes = [K0] + ([K1] if K1 else [])

    nc = bacc.Bacc("TRN2", target_bir_lowering=False, debug=False,
                   num_devices=NCORES)
    din = {}
    for name, shape, dt in [
            ("xT", [IN, NTP], BF), ("xTo", [IN, NODES_PAD], BF),
            ("dstat", [HID, 2], F32), ("S_all", [128, NODES_PAD], BF),
            ("esrc1", [128, NSUB], I32), ("esrc2", [128, NSUB], I32),
            ("Wl1", [IN, F1], BF), ("Wr1", [IN, F1], BF),
            ("Wl2", [HID, F2], BF), ("Wr2", [HID, F2], BF),
            ("A1", [F1, HEADS], BF), ("A2", [F2, HEADS], BF),
            ("gb1", [IN, 2], F32), ("gb2", [HID, 2], F32),
            ("b1c", [HID, 1], F32), ("b2c", [OUT, 1], F32),
            ("mean1m", [128, HID], BF), ("mean2m", [128, OUT], BF),
            ("e1m", [8, F1], BF), ("e2m", [8, F2], BF)]:
        din[name] = nc.dram_tensor(name, shape, dt, kind="ExternalInput")
    outT = nc.dram_tensor("outT", [OUT, NODES_PAD], F32, kind="ExternalOutput")

    xl1_full = nc.dram_tensor("xl1_full", [NTP, F1], BF)
    xl2_full = nc.dram_tensor("xl2_full", [NCORES * NODES_PAD + 128, F2], BF)
    st1_in = nc.dram_tensor("st1_in", [IN, 2], F32)
    st1_out = nc.dram_tensor("st1_out", [IN, 2], F32)
    ag_in = nc.dram_tensor("ag_in", [HID + 2, NODES_PAD], BF)
    ag_out = nc.dram_tensor("ag_out", [NCORES * (HID + 2), NODES_PAD], BF,
                            addr_space="Shared")

    import contextlib
    with tile.TileContext(nc) as tc:
        ctx = contextlib.ExitStack()
        with ctx:
            cpool = ctx.enter_context(tc.tile_pool(name="const", bufs=1))
            rpool = ctx.enter_context(tc.tile_pool(name="resident", bufs=1))

            # ---------- constants / resident inputs ----------
            ident = cpool.tile([128, 128], BF)
            make_identity(nc, ident[:])
            epsb = cpool.tile([128, 1], F32, tag="epsb")
            nc.vector.memset(epsb[:], BN_EPS)
            msh1 = cpool.tile([128, 1], F32, tag="msh1")
            nc.vector.memset(msh1[:], -float(cfg.M1))
            msh2 = cpool.tile([128, 1], F32, tag="msh2")
            nc.vector.memset(msh2[:], -float(cfg.M2))
            nslope = cpool.tile([128, 1], F32, tag="nslope")
            nc.vector.memset(nslope[:], -(1.0 - NEG_SLOPE))

            def load_res(name, shape, dt=BF):
                t = rpool.tile(shape, dt, tag=name, name=name)
                nc.sync.dma_start(out=t[:], in_=din[name].ap())
                return t

            S_sb = load_res("S_all", [128, NODES_PAD])
            esrc1_sb = load_res("esrc1", [128, NSUB], I32)
            esrc2_sb = load_res("esrc2", [128, NSUB], I32)
            a1h, a2h = [], []
            for h in range(F1 // 128):
                t = rpool.tile([128, HEADS], BF, tag=f"A1_{h}",
                               name=f"A1_{h}")
                nc.sync.dma_start(
                    out=t[:], in_=din["A1"].ap()[h * 128:(h + 1) * 128, :])
                a1h.append(t)
            for h in range(F2 // 128):
                t = rpool.tile([128, HEADS], BF, tag=f"A2_{h}",
                               name=f"A2_{h}")
                nc.sync.dma_start(
                    out=t[:], in_=din["A2"].ap()[h * 128:(h + 1) * 128, :])
                a2h.append(t)
            mean1 = load_res("mean1m", [128, HID])
            mean2 = load_res("mean2m", [128, OUT])
            e1full = load_res("e1m", [8, F1])
            e2full = load_res("e2m", [8, F2])
            e1h = [e1full[:, h * 128:(h + 1) * 128] for h in range(F1 // 128)]
            e2h = [e2full[:, h * 128:(h + 1) * 128] for h in range(F2 // 128)]
            b1c_sb = load_res("b1c", [HID, 1], F32)
            b2c_sb = load_res("b2c", [OUT, 1], F32)
            dstat_sb = load_res("dstat", [HID, 2], F32)
            wl1_sb, wr1_sb = [], []
            for ki, ks in enumerate(kt_sizes):
                for wname, lst in (("Wl1", wl1_sb), ("Wr1", wr1_sb)):
                    t = rpool.tile([ks, F1], BF, tag=f"{wname}_{ki}",
                                   name=f"{wname}_{ki}")
                    nc.sync.dma_start(
                        out=t[:],
                        in_=din[wname].ap()[ki * 128:ki * 128 + ks, :])
                    lst.append(t)
            wl2_sb = load_res("Wl2", [HID, F2])
            wr2_sb = load_res("Wr2", [HID, F2])

            # ---------- phase A: BN1 stats + AllReduce ----------
            pXT_cm = tc.tile_pool(name="pXT", bufs=1)
            pXT = pXT_cm.__enter__()
            xTo_sb = []
            for ki, ks in enumerate(kt_sizes):
                t = pXT.tile([ks, NODES_PAD], BF, tag=f"xTo{ki}",
                             name=f"xTo{ki}")
                nc.sync.dma_start(
                    out=t[:], in_=din["xTo"].ap()[ki * 128:ki * 128 + ks, :])
                xTo_sb.append(t)
            with tc.tile_pool(name="pA", bufs=2) as pa, \
                 tc.tile_pool(name="pAs", bufs=1) as pas:
                for ki, ks in enumerate(kt_sizes):
                    st = pa.tile([ks, 2], F32, tag="st")
                    nc.vector.tensor_reduce(out=st[:, 0:1], in_=xTo_sb[ki][:],
                                            axis=mybir.AxisListType.X,
                                            op=mybir.AluOpType.add)
                    scr = pas.tile([ks, NODES_PAD], BF, tag="scr")
                    nc.scalar.activation(
                        out=scr[:], in_=xTo_sb[ki][:],
                        func=mybir.ActivationFunctionType.Square,
                        accum_out=st[:, 1:2])
                    nc.sync.dma_start(
                        out=st1_in.ap()[ki * 128:ki * 128 + ks, :], in_=st[:])
            nc.gpsimd.collective_compute(
                "AllReduce", mybir.AluOpType.add,
                ins=[st1_in.ap()], outs=[st1_out.ap()],
                replica_groups=[list(range(NCORES))])

            # fold stats -> s1, t1 per K tile (applied as Act scale/bias)
            s1_t, t1_t = [], []
            with tc.tile_pool(name="pB", bufs=1) as pb:
                for ki, ks in enumerate(kt_sizes):
                    stg = pb.tile([ks, 2], F32, tag=f"stg{ki}", name=f"stg{ki}")
                    nc.sync.dma_start(
                        out=stg[:],
                        in_=st1_out.ap()[ki * 128:ki * 128 + ks, :])
                    gb = pb.tile([ks, 2], F32, tag=f"gb{ki}", name=f"gb{ki}")
                    nc.sync.dma_start(
                        out=gb[:],
                        in_=din["gb1"].ap()[ki * 128:ki * 128 + ks, :])
                    mean = pb.tile([ks, 1], F32, tag=f"mean{ki}",
                                   name=f"mean{ki}")
                    nc.vector.tensor_scalar(out=mean[:], in0=stg[:, 0:1],
                                            scalar1=RECIP_N, scalar2=None,
                                            op0=mybir.AluOpType.mult)
                    q = pb.tile([ks, 1], F32, tag=f"q{ki}", name=f"q{ki}")
                    nc.vector.tensor_scalar(out=q[:], in0=stg[:, 1:2],
                                            scalar1=RECIP_N, scalar2=None,
                                            op0=mybir.AluOpType.mult)
                    m2 = pb.tile([ks, 1], F32, tag=f"m2{ki}", name=f"m2{ki}")
                    nc.vector.tensor_tensor(out=m2[:], in0=mean[:],
                                            in1=mean[:],
                                            op=mybir.AluOpType.mult)
                    var = pb.tile([ks, 1], F32, tag=f"var{ki}",
                                  name=f"var{ki}")
                    nc.vector.tensor_tensor(out=var[:], in0=q[:], in1=m2[:],
                                            op=mybir.AluOpType.subtract)
                    sd = pb.tile([ks, 1], F32, tag=f"sd{ki}", name=f"sd{ki}")
                    nc.scalar.activation(
                        out=sd[:], in_=var[:],
                        func=mybir.ActivationFunctionType.Sqrt,
                        bias=epsb[:ks, :1])
                    rstd = pb.tile([ks, 1], F32, tag=f"rstd{ki}",
                                   name=f"rstd{ki}")
                    nc.vector.reciprocal(rstd[:], sd[:])
                    s1 = rpool.tile([ks, 1], F32, tag=f"s1_{ki}",
                                    name=f"s1_{ki}")
                    nc.vector.tensor_tensor(out=s1[:], in0=gb[:, 0:1],
                                            in1=rstd[:],
                                            op=mybir.AluOpType.mult)
                    ms = pb.tile([ks, 1], F32, tag=f"ms{ki}", name=f"ms{ki}")
                    nc.vector.tensor_tensor(out=ms[:], in0=mean[:], in1=s1[:],
                                            op=mybir.AluOpType.mult)
                    t1 = rpool.tile([ks, 1], F32, tag=f"t1_{ki}",
                                    name=f"t1_{ki}")
                    nc.vector.tensor_tensor(out=t1[:], in0=gb[:, 1:2],
                                            in1=ms[:],
                                            op=mybir.AluOpType.subtract)
                    s1_t.append(s1)
                    t1_t.append(t1)

            # ---------- phase A2: xn_own + xr1T resident ----------
            xn_own = []
            for ki, ks in enumerate(kt_sizes):
                t = pXT.tile([ks, NODES_PAD], BF, tag=f"xn{ki}",
                             name=f"xn{ki}")
                nc.scalar.activation(
                    out=t[:], in_=xTo_sb[ki][:],
                    func=mybir.ActivationFunctionType.Identity,
                    scale=s1_t[ki][:, :1], bias=t1_t[ki][:, :1])
                xn_own.append(t)
            nh1 = F1 // 128
            XRPAD = NODES_PAD + 128
            xr1T = [rpool.tile([128, XRPAD], BF, tag=f"xr1T_{h}",
                               name=f"xr1T_{h}") for h in range(nh1)]
            GB = 4   # groups per psum bank
            with tc.tile_pool(name="pC", bufs=2, space="PSUM") as pc:
                for h in range(nh1):
                    nc.vector.memset(xr1T[h][:, NODES_PAD:], 0.0)
                for g0 in range(0, NG, GB):
                    gn = min(GB, NG - g0)
                    for h in range(nh1):
                        ps = pc.tile([128, GB * 128], F32, space="PSUM",
                                     tag="xr1p")
                        for gi in range(gn):
                            cols = slice((g0 + gi) * 128, (g0 + gi + 1) * 128)
                            for ki, ks in enumerate(kt_sizes):
                                nc.tensor.matmul(
                                    out=ps[:, gi * 128:(gi + 1) * 128],
                                    lhsT=wr1_sb[ki][:, h * 128:(h + 1) * 128],
                                    rhs=xn_own[ki][:, cols],
                                    start=(ki == 0),
                                    stop=(ki == len(kt_sizes) - 1))
                        dstc = xr1T[h][:, g0 * 128:(g0 + gn) * 128]
                        if h % 2 == 0:
                            nc.vector.tensor_copy(out=dstc,
                                                  in_=ps[:, :gn * 128])
                        else:
                            nc.scalar.copy(out=dstc, in_=ps[:, :gn * 128])

            pXT_cm.__exit__(None, None, None)

            # ---------- phase A3: xl1_full table (batched) ----------
            TB = 8   # node tiles per batch
            n_t1 = NTP // 128
            with tc.tile_pool(name="pD", bufs=3) as pd, \
                 tc.tile_pool(name="pDp", bufs=2, space="PSUM") as pdp:
                for t0 in range(0, n_t1, TB):
                    tn = min(TB, n_t1 - t0)
                    cols = slice(t0 * 128, (t0 + tn) * 128)
                    lhs = []
                    for ki, ks in enumerate(kt_sizes):
                        lt = pd.tile([ks, TB * 128], BF, tag=f"xl1l{ki}",
                                     name=f"xl1l{ki}")
                        nc.sync.dma_start(
                            out=lt[:, :tn * 128],
                            in_=din["xT"].ap()[ki * 128:ki * 128 + ks, cols])
                        ln = pd.tile([ks, TB * 128], BF, tag=f"xl1n{ki}",
                                     name=f"xl1n{ki}")
                        nc.scalar.activation(
                            out=ln[:, :tn * 128], in_=lt[:, :tn * 128],
                            func=mybir.ActivationFunctionType.Identity,
                            scale=s1_t[ki][:, :1], bias=t1_t[ki][:, :1])
                        lhs.append(ln)
                    ob = pd.tile([128, TB * F1], BF, tag="xl1o", name="xl1o")
                    ps_cur = None
                    for ti in range(tn):
                        half = ti % 2
                        if half == 0:
                            ps_cur = pdp.tile([128, 2 * F1], F32,
                                              space="PSUM", tag="xl1p",
                                              name="xl1p")
                        pslice = ps_cur[:, half * F1:(half + 1) * F1]
                        for ki, ks in enumerate(kt_sizes):
                            nc.tensor.matmul(
                                out=pslice,
                                lhsT=lhs[ki][:, ti * 128:(ti + 1) * 128],
                                rhs=wl1_sb[ki][:],
                                start=(ki == 0),
                                stop=(ki == len(kt_sizes) - 1))
                        if half == 1 or ti == tn - 1:
                            nf = (half + 1) * F1
                            o0 = (ti - half) * F1
                            if ti % 4 < 2:
                                nc.vector.tensor_copy(
                                    out=ob[:, o0:o0 + nf],
                                    in_=ps_cur[:, :nf])
                            else:
                                nc.scalar.copy(
                                    out=ob[:, o0:o0 + nf],
                                    in_=ps_cur[:, :nf])
                    nc.sync.dma_start(
                        out=xl1_full.ap()[t0 * 128:(t0 + tn) * 128, :]
                            .rearrange("(t p) f -> p t f", p=128),
                        in_=ob[:, :tn * F1].rearrange("p (t f) -> p t f",
                                                      f=F1))

            with tc.tile_pool(name="pZ1", bufs=1) as pz1:
                zrow = pz1.tile([1, F1], BF, tag="zrow")
                nc.vector.memset(zrow[:], 0.0)
                nc.sync.dma_start(out=xl1_full.ap()[NTP - 1:NTP, :],
                                  in_=zrow[:])

            # ---------- edge phase helper ----------
            GATHK = 1   # HW indirect DMA: one offset per partition row

            def edge_phase(F, xfull, esrc_sb, ah, eh, meanm, shift_ap,
                           bias_col, out_cb, xr_list, layer):
                nhalf = F // 128
                groups = {}
                for s, (d, take, nstart) in enumerate(subtiles):
                    groups.setdefault(nstart // 128, []).append(
                        (s, d, take, nstart))
                ggp_bufs = 12 if GATHK <= 2 else (6 if GATHK <= 6 else 3)
                with tc.tile_pool(name=f"gg{layer}", bufs=ggp_bufs) as ggp, \
                     tc.tile_pool(name=f"gz{layer}", bufs=2,
                                  space="PSUM") as gz, \
                     tc.tile_pool(name=f"gl{layer}", bufs=1,
                                  space="PSUM") as gl, \
                     tc.tile_pool(name=f"gn{layer}", bufs=1,
                                  space="PSUM") as gn, \
                     tc.tile_pool(name=f"gs{layer}", bufs=4) as gs:
                    xls_t = {}
                    for s0 in range(0, NSUB, GATHK):
                        sn = min(GATHK, NSUB - s0)
                        xg = ggp.tile([128, GATHK * F], BF, tag="xg",
                                      name="xg")
                        nc.gpsimd.indirect_dma_start(
                            out=xg[:, :sn * F], out_offset=None,
                            in_=xfull.ap(),
                            in_offset=bass.IndirectOffsetOnAxis(
                                ap=esrc_sb[:, s0:s0 + sn], axis=0))
                        for j in range(sn):
                            xls_t[s0 + j] = (xg, j * F)
                    for g in range(NG):
                        subs = groups[g]
                        numT = gn.tile([128, nhalf * 128], F32, space="PSUM",
                                       tag="numT", name="numT")
                        denT = gn.tile([8, 128], F32, space="PSUM",
                                       tag="denT", name="denT")
                        cov = max(st[3] % 128 + st[2] for st in subs)
                        for b0 in range(0, len(subs), 4):
                            batch = subs[b0:b0 + 4]
                            nb = len(batch)
                            zts = []
                            for h in range(nhalf):
                                zt = gz.tile([128, 4 * 128], F32,
                                             space="PSUM", tag=f"zt{h}",
                                             name=f"zt{h}")
                                zts.append(zt)
                            lg = gl.tile([128, 4 * 8], F32, space="PSUM",
                                         tag="lg", name="lg")
                            for si, (s, d, take, nstart) in enumerate(batch):
                                xg, xo = xls_t[s]
                                ncov = take * d
                                for h in range(nhalf):
                                    zsl = zts[h][:, si * 128:(si + 1) * 128]
                                    xgh = xg[:, xo + h * 128:
                                             xo + (h + 1) * 128]
                                    xr_t = xr_list[h]
                                    rep = xr_t[:, nstart:nstart + take, None] \
                                        .broadcast_to([128, take, d])
                                    if ncov == 128:
                                        nc.tensor.matmul(
                                            out=zsl, lhsT=xgh, rhs=ident[:],
                                            start=True, stop=False)
                                        nc.tensor.matmul(
                                            out=zsl, lhsT=ident[:], rhs=rep,
                                            start=False, stop=True)
                                    else:
                                        zla = zts[h][:, si * 128:
                                                     si * 128 + ncov]
                                        zlb = zts[h][:, si * 128 + ncov:
                                                     (si + 1) * 128]
                                        nc.tensor.matmul(
                                            out=zla, lhsT=xgh,
                                            rhs=ident[:, 0:ncov],
                                            start=True, stop=False)
                                        nc.tensor.matmul(
                                            out=zla, lhsT=ident[:], rhs=rep,
                                            start=False, stop=True)
                                        rep2 = xr_t[:, nstart + take:
                                                    nstart + take + 1, None] \
                                            .broadcast_to([128, 1, 128 - ncov])
                                        nc.tensor.matmul(
                                            out=zlb, lhsT=xgh,
                                            rhs=ident[:, ncov:128],
                                            start=True, stop=False)
                                        nc.tensor.matmul(
                                            out=zlb, lhsT=ident[:], rhs=rep2,
                                            start=False, stop=True)
                            es = []
                            for h in range(nhalf):
                                r8 = gs.tile([128, 4 * 128], BF,
                                             tag=f"r8{h}", name=f"r8{h}")
                                nc.scalar.activation(
                                    out=r8[:, :nb * 128],
                                    in_=zts[h][:, :nb * 128],
                                    func=mybir.ActivationFunctionType.Relu,
                                    scale=nslope[:, :1])
                                e_sb = gs.tile([128, 4 * 128], BF,
                                               tag=f"es{h}", name=f"es{h}")
                                nc.vector.tensor_tensor(
                                    out=e_sb[:, :nb * 128],
                                    in0=zts[h][:, :nb * 128],
                                    in1=r8[:, :nb * 128],
                                    op=mybir.AluOpType.add)
                                es.append(e_sb)
                            for si in range(nb):
                                for h in range(nhalf):
                                    nc.tensor.matmul(
                                        out=lg[:, si * 8:(si + 1) * 8],
                                        lhsT=es[h][:, si * 128:(si + 1) * 128],
                                        rhs=ah[h][:, :],
                                        start=(h == 0), stop=(h == nhalf - 1))
                            w4 = gs.tile([128, 4 * 8], BF, tag="w4",
                                         name="w4")
                            nc.scalar.activation(
                                out=w4[:, :nb * 8], in_=lg[:, :nb * 8],
                                func=mybir.ActivationFunctionType.Exp,
                                bias=shift_ap[:, :1])
                            y4 = gs.tile([128, 4 * F], BF, tag="y4",
                                         name="y4")
                            for si, (s, d, take, nstart) in enumerate(batch):
                                xg, xo = xls_t[s]
                                wv = w4[:, si * 8:(si + 1) * 8, None] \
                                    .broadcast_to([128, 8, F // 8])
                                nc.vector.tensor_tensor(
                                    out=y4[:, si * F:(si + 1) * F].rearrange(
                                        "p (a b) -> p a b", a=8),
                                    in0=xg[:, xo:xo + F].rearrange(
                                        "p (a b) -> p a b", a=8),
                                    in1=wv, op=mybir.AluOpType.mult)
                            for si, (s, d, take, nstart) in enumerate(batch):
                                noff = nstart % 128
                                for h in range(nhalf):
                                    nc.tensor.matmul(
                                        out=numT[:, h * 128 + noff:
                                                 h * 128 + noff + take],
                                        lhsT=y4[:, si * F + h * 128:
                                                si * F + (h + 1) * 128],
                                        rhs=S_sb[:, nstart:nstart + take],
                                        start=True, stop=True)
                                nc.tensor.matmul(
                                    out=denT[0:8, noff:noff + take],
                                    lhsT=w4[:, si * 8:(si + 1) * 8],
                                    rhs=S_sb[:, nstart:nstart + take],
                                    start=True, stop=True)
                        if cov < 128:
                            zc = slice(g * 128 + cov, (g + 1) * 128)
                            for h in range(nhalf):
                                nc.tensor.matmul(
                                    out=numT[:, h * 128 + cov:h * 128 + 128],
                                    lhsT=ident[:], rhs=S_sb[:, zc],
                                    start=True, stop=True)
                            nc.tensor.matmul(
                                out=denT[0:8, cov:128],
                                lhsT=ident[:, 0:8], rhs=S_sb[:, zc],
                                start=True, stop=True)
                        # ---- group epilogue ----
                        den_s = gs.tile([8, 128], F32, tag="den_s",
                                        name="den_s")
                        nc.vector.tensor_scalar(
                            out=den_s[:], in0=denT[:], scalar1=1e-30,
                            scalar2=None, op0=mybir.AluOpType.max)
                        drec = gs.tile([8, 128], F32, tag="drec", name="drec")
                        nc.vector.reciprocal(drec[:], den_s[:])
                        drecb = gs.tile([8, 128], BF, tag="drecb",
                                        name="drecb")
                        nc.vector.tensor_copy(out=drecb[:], in_=drec[:])
                        onts = []
                        for h in range(nhalf):
                            rexp = gz.tile([128, 4 * 128], F32, space="PSUM",
                                           tag=f"zt{h}", name=f"rexp{h}")
                            nc.tensor.matmul(out=rexp[:, :128], lhsT=eh[h],
                                             rhs=drecb[:], start=True,
                                             stop=True)
                            rexpb = gs.tile([128, 128], BF, tag=f"rexpb{h}",
                                            name=f"rexpb{h}")
                            nc.scalar.copy(out=rexpb[:], in_=rexp[:, :128])
                            ont = gs.tile([128, 128], BF, tag=f"ont{h}",
                                          name=f"ont{h}")
                            nc.vector.tensor_tensor(
                                out=ont[:],
                                in0=numT[:, h * 128:(h + 1) * 128],
                                in1=rexpb[:], op=mybir.AluOpType.mult)
                            onts.append(ont)
                        cdim = HID if layer == 1 else OUT
                        ot = gl.tile([cdim, 128], F32, space="PSUM",
                                     tag="lg", name="otp")
                        for h in range(nhalf):
                            nc.tensor.matmul(out=ot[:], lhsT=meanm[:, :cdim],
                                             rhs=onts[h][:], start=(h == 0),
                                             stop=(h == nhalf - 1))
                        out_cb(g, ot, bias_col)

            # ---------- phase B: layer-1 edges -> h1T ----------
            h1T = rpool.tile([HID, NODES_PAD], BF)
            oB = ctx.enter_context(tc.tile_pool(name="oB", bufs=2))

            def l1_out(g, ot_psum, bias_col):
                nc.scalar.activation(out=h1T[:, g * 128:(g + 1) * 128],
                                     in_=ot_psum[:],
                                     func=mybir.ActivationFunctionType.Relu,
                                     bias=bias_col[:, :1])

            edge_phase(F1, xl1_full, esrc1_sb, a1h, e1h, mean1, msh1,
                       b1c_sb, l1_out, xr1T, layer=1)

            # ---------- phase C: AllGather h1T + BN2 + xl2 + xr2T ----------
            with tc.tile_pool(name="pE", bufs=2) as pe:
                st2 = pe.tile([HID, 2], F32, tag="st2")
                nc.vector.tensor_reduce(out=st2[:, 0:1], in_=h1T[:],
                                        axis=mybir.AxisListType.X,
                                        op=mybir.AluOpType.add)
                scr2 = pe.tile([HID, NODES_PAD], BF, tag="scr2")
                nc.scalar.activation(
                    out=scr2[:], in_=h1T[:],
                    func=mybir.ActivationFunctionType.Square,
                    accum_out=st2[:, 1:2])
                nc.vector.tensor_tensor(out=st2[:], in0=st2[:],
                                        in1=dstat_sb[:],
                                        op=mybir.AluOpType.subtract)
                nc.sync.dma_start(out=ag_in.ap()[0:HID, :], in_=h1T[:])
                nc.sync.dma_start(out=ag_in.ap()[HID:HID + 1, 0:2 * HID],
                                  in_=st2[:, 0:1].bitcast(BF))
                nc.sync.dma_start(out=ag_in.ap()[HID + 1:HID + 2, 0:2 * HID],
                                  in_=st2[:, 1:2].bitcast(BF))
            nc.gpsimd.collective_compute(
                "AllGather", mybir.AluOpType.bypass,
                ins=[ag_in.ap()], outs=[ag_out.ap()],
                replica_groups=[list(range(NCORES))])

            with tc.tile_pool(name="pF", bufs=1) as pf:
                s2sum = pf.tile([HID, NCORES], F32, tag="s2sum")
                s2sq = pf.tile([HID, NCORES], F32, tag="s2sq")
                agf = ag_out.ap().bitcast(F32)
                for c in range(NCORES):
                    r = c * (HID + 2) + HID
                    nc.sync.dma_start(out=s2sum[:, c:c + 1],
                                      in_=agf[r:r + 1, 0:HID])
                    nc.sync.dma_start(out=s2sq[:, c:c + 1],
                                      in_=agf[r + 1:r + 2, 0:HID])
                stg = pf.tile([HID, 2], F32, tag="stg2")
                nc.vector.tensor_reduce(out=stg[:, 0:1], in_=s2sum[:],
                                        axis=mybir.AxisListType.X,
                                        op=mybir.AluOpType.add)
                nc.vector.tensor_reduce(out=stg[:, 1:2], in_=s2sq[:],
                                        axis=mybir.AxisListType.X,
                                        op=mybir.AluOpType.add)
                gb = pf.tile([HID, 2], F32, tag="gb2")
                nc.sync.dma_start(out=gb[:], in_=din["gb2"].ap())
                mean = pf.tile([HID, 1], F32, tag="mean2")
                nc.vector.tensor_scalar(out=mean[:], in0=stg[:, 0:1],
                                        scalar1=RECIP_N, scalar2=None,
                                        op0=mybir.AluOpType.mult)
                q = pf.tile([HID, 1], F32, tag="q2")
                nc.vector.tensor_scalar(out=q[:], in0=stg[:, 1:2],
                                        scalar1=RECIP_N, scalar2=None,
                                        op0=mybir.AluOpType.mult)
                m2 = pf.tile([HID, 1], F32, tag="m22")
                nc.vector.tensor_tensor(out=m2[:], in0=mean[:], in1=mean[:],
                                        op=mybir.AluOpType.mult)
                var = pf.tile([HID, 1], F32, tag="var2")
                nc.vector.tensor_tensor(out=var[:], in0=q[:], in1=m2[:],
                                        op=mybir.AluOpType.subtract)
                sd = pf.tile([HID, 1], F32, tag="sd2")
                nc.scalar.activation(out=sd[:], in_=var[:],
                                     func=mybir.ActivationFunctionType.Sqrt,
                                     bias=epsb[:HID, :1])
                rstd = pf.tile([HID, 1], F32, tag="rstd2")
                nc.vector.reciprocal(rstd[:], sd[:])
                s2c = rpool.tile([HID, 1], F32, tag="s2c", name="s2c")
                nc.vector.tensor_tensor(out=s2c[:], in0=gb[:, 0:1],
                                        in1=rstd[:],
                                        op=mybir.AluOpType.mult)
                ms = pf.tile([HID, 1], F32, tag="ms2")
                nc.vector.tensor_tensor(out=ms[:], in0=mean[:], in1=s2c[:],
                                        op=mybir.AluOpType.mult)
                t2c = rpool.tile([HID, 1], F32, tag="t2c", name="t2c")
                nc.vector.tensor_tensor(out=t2c[:], in0=gb[:, 1:2], in1=ms[:],
                                        op=mybir.AluOpType.subtract)

            # xl2_full table (batched, bn via Act on gathered h)
            TB2 = 8
            n_t2 = NODES_PAD // 128
            with tc.tile_pool(name="pG", bufs=3) as pg, \
                 tc.tile_pool(name="pGp", bufs=2, space="PSUM") as pgp:
                for c_src in range(NCORES):
                    r0 = c_src * (HID + 2)
                    for t0 in range(0, n_t2, TB2):
                        tn = min(TB2, n_t2 - t0)
                        cols = slice(t0 * 128, (t0 + tn) * 128)
                        lt = pg.tile([HID, TB2 * 128], BF, tag="xl2l",
                                     name="xl2l")
                        nc.sync.dma_start(out=lt[:, :tn * 128],
                                          in_=ag_out.ap()[r0:r0 + HID, cols])
                        ln = pg.tile([HID, TB2 * 128], BF, tag="xl2n",
                                     name="xl2n")
                        nc.scalar.activation(
                            out=ln[:, :tn * 128], in_=lt[:, :tn * 128],
                            func=mybir.ActivationFunctionType.Identity,
                            scale=s2c[:, :1], bias=t2c[:, :1])
                        ob = pg.tile([128, TB2 * F2], BF, tag="xl2o",
                                     name="xl2o")
                        for ti in range(0, tn, 4):
                            t4 = min(4, tn - ti)
                            ps = pgp.tile([128, 4 * F2], F32, space="PSUM",
                                          tag="xl2p", name="xl2p")
                            for tj in range(t4):
                                nc.tensor.matmul(
                                    out=ps[:, tj * F2:(tj + 1) * F2],
                                    lhsT=ln[:, (ti + tj) * 128:
                                            (ti + tj + 1) * 128],
                                    rhs=wl2_sb[:], start=True, stop=True)
                            o0 = ti * F2
                            if (ti // 4) % 2 == 0:
                                nc.vector.tensor_copy(
                                    out=ob[:, o0:o0 + t4 * F2],
                                    in_=ps[:, :t4 * F2])
                            else:
                                nc.scalar.copy(
                                    out=ob[:, o0:o0 + t4 * F2],
                                    in_=ps[:, :t4 * F2])
                        base = c_src * NODES_PAD + t0 * 128
                        nc.sync.dma_start(
                            out=xl2_full.ap()[base:base + tn * 128, :]
                                .rearrange("(t p) f -> p t f", p=128),
                            in_=ob[:, :tn * F2].rearrange("p (t f) -> p t f",
                                                          f=F2))

            with tc.tile_pool(name="pZ2", bufs=1) as pz2:
                zrow2 = pz2.tile([1, F2], BF, tag="zrow2")
                nc.vector.memset(zrow2[:], 0.0)
                nc.sync.dma_start(
                    out=xl2_full.ap()[NCORES * NODES_PAD:
                                      NCORES * NODES_PAD + 1, :],
                    in_=zrow2[:])
            # h1n (scoped) + xr2T resident
            pHN_cm = tc.tile_pool(name="pHN", bufs=1)
            pHN = pHN_cm.__enter__()
            h1n = pHN.tile([HID, NODES_PAD], BF)
            nc.scalar.activation(out=h1n[:], in_=h1T[:],
                                 func=mybir.ActivationFunctionType.Identity,
                                 scale=s2c[:, :1], bias=t2c[:, :1])
            xr2T_t = rpool.tile([128, XRPAD], BF, tag="xr2T", name="xr2T")
            nc.vector.memset(xr2T_t[:, NODES_PAD:], 0.0)
            with tc.tile_pool(name="pH", bufs=2, space="PSUM") as ph:
                for g0 in range(0, NG, GB):
                    gn = min(GB, NG - g0)
                    ps = ph.tile([128, GB * 128], F32, space="PSUM",
                                 tag="xr2p")
                    for gi in range(gn):
                        cols = slice((g0 + gi) * 128, (g0 + gi + 1) * 128)
                        nc.tensor.matmul(out=ps[:, gi * 128:(gi + 1) * 128],
                                         lhsT=wr2_sb[:], rhs=h1n[:, cols],
                                         start=True, stop=True)
                    if (g0 // GB) % 2 == 0:
                        nc.vector.tensor_copy(
                            out=xr2T_t[:, g0 * 128:(g0 + gn) * 128],
                            in_=ps[:, :gn * 128])
                    else:
                        nc.scalar.copy(
                            out=xr2T_t[:, g0 * 128:(g0 + gn) * 128],
                            in_=ps[:, :gn * 128])

            # ---------- phase D: layer-2 edges -> outT ----------
            pHN_cm.__exit__(None, None, None)
            oD = ctx.enter_context(tc.tile_pool(name="oD", bufs=2))
            OGB = 8
            ost = {}

            def l2_out(g, ot_psum, bias_col):
                if g % OGB == 0:
                    ost["t"] = oD.tile([OUT, OGB * 128], F32, tag="ob",
                                       name="ob")
                    ost["g0"] = g
                j = g - ost["g0"]
                nc.scalar.activation(
                    out=ost["t"][:, j * 128:(j + 1) * 128], in_=ot_psum[:],
                    func=mybir.ActivationFunctionType.Identity,
                    bias=bias_col[:, :1])
                if j == OGB - 1 or g == NG - 1:
                    nc.sync.dma_start(
                        out=outT.ap()[:, ost["g0"] * 128:(g + 1) * 128],
                        in_=ost["t"][:, :(j + 1) * 128])

            edge_phase(F2, xl2_full, esrc2_sb, a2h, e2h, mean2, msh2,
                       b2c_sb, l2_out, [xr2T_t], layer=2)

    nc.compile()
    return nc


_CACHE = {}


def _get_nc(cfg, meta):
    key = (cfg.N, cfg.IN, cfg.HID, cfg.OUT, meta["NSUB"], meta["NODES_PAD"])
    if key not in _CACHE:
        _CACHE[key] = _build(cfg, meta)
    return _CACHE[key]


def run(cfg, inputs):
    x = np.asarray(inputs["x"], np.float32)
    ei = np.asarray(inputs["edge_index"], np.int32)
    W = {k: np.asarray(inputs[k], np.float32) for k in
         ("Wl1", "Wr1", "att1", "b1", "gamma1", "beta1",
          "Wl2", "Wr2", "att2", "b2", "gamma2", "beta2")}
    meta = _preprocess(cfg, x, ei, W)
    nc = _get_nc(cfg, meta)
    res = run_bass_kernel_spmd(nc, meta["in_maps"],
                               core_ids=list(range(NCORES)))
    out = np.empty((cfg.N, cfg.OUT), np.float32)
    proc = meta["proc"]
    for c in range(NCORES):
        oT = res.results[c]["outT"]      # [OUT, NODES_PAD]
        sel = proc[c] >= 0
        out[proc[c][sel]] = oT[:, sel].T
    return out, meta, nc


def kernel(**inputs):
    cfg = Cfg(50000, 200, 32, 16, m1=8.0, m2=10.0)
    out, _, _ = run(cfg, inputs)
    return out


# revision 19
# speedup vs baseline: 60.1224x; 3.9112x over previous
"""Distributed GATv2 (2 layers + BN) Bass kernel for 8 trn2 NeuronCores. v2

Strategy: nodes dealt round-robin by degree across 8 cores (dst-ownership,
degree-balanced so the SPMD subtile schedule is identical on every core).
Each core:
  - BN1 stats partials on own nodes -> AllReduce -> s1/t1 applied via
    Act-engine scale/bias on x tiles (no weight folding)
  - xl1 table = bn(x)@Wl1 for ALL nodes (bf16 DRAM, batched DMA)
  - xr1T = Wr1^T @ bn(x_own) feat-major resident
  - edge phase L1: subtiles of 128 edge slots, exact-degree packing
    (nps nodes x d edges, tail slots dead). Batched indirect gather of
    xl1[src] (16 subtiles/DMA); per subtile: PE transpose + xr broadcast
    add -> z (PSUM), one DVE scalar_tensor_tensor LeakyReLU, PE logits vs
    block-diag att, Act exp, DVE alpha-weighting, PE one-hot
    (host-precomputed S_all) segment sums numT/denT, group epilogue with
    clamped-denominator reciprocal -> h1T
  - AllGather h1T (+BN2 stats rows) -> BN2 via Act scale/bias -> xl2
    table + xr2T -> edge phase L2 -> outT (single store)
Output per core: outT [16, NODES_PAD] f32; host unpermutes via proc map.
"""
import sys
import numpy as np

sys.path.insert(0, "/opt/trn_rl_repo")

import concourse.bass as bass          # noqa: E402
import concourse.bacc as bacc          # noqa: E402
import concourse.tile as tile          # noqa: E402
from concourse import mybir            # noqa: E402
from concourse.bass_utils import run_bass_kernel_spmd  # noqa: E402
from concourse.masks import make_identity  # noqa: E402

F32 = mybir.dt.float32
BF = mybir.dt.bfloat16
I32 = mybir.dt.int32
NPBF = mybir.dt.np(BF)

NCORES = 8
HEADS = 8
BN_EPS = 1e-5
NEG_SLOPE = 0.2


class Cfg:
    def __init__(self, n_nodes, in_dim, hid, out, m1, m2):
        self.N = n_nodes
        self.IN = in_dim
        self.HID = hid
        self.OUT = out
        self.F1 = HEADS * hid
        self.F2 = HEADS * out
        self.M1 = m1          # logit shift (softmax-invariant), layer 1
        self.M2 = m2
        self.NT_PAD = ((n_nodes + 127) // 128) * 128
        self.KT = [min(128, in_dim), max(0, in_dim - 128)]  # K tiles for IN


def _schedule_v2(dmax):
    """Exact-degree subtile schedule from cross-core max per-degree counts.
    Returns subtiles [(d, take, nstart)], NODES_PAD, NSUB."""
    subtiles = []
    pos = 0
    for d in sorted(dmax):
        cnt = dmax[d]
        nps = 128 // d
        while cnt > 0:
            take = min(nps, cnt)
            cnt -= take
            if pos // 128 != (pos + take - 1) // 128:
                pos = ((pos // 128) + 1) * 128
            subtiles.append((d, take, pos))
            pos += take
    nodes_pad = ((pos + 127) // 128) * 128
    return subtiles, nodes_pad, len(subtiles)


def _preprocess(cfg, x, edge_index, W):
    N = cfg.N
    src = np.concatenate([edge_index[0], np.arange(N, dtype=np.int32)])
    dst = np.concatenate([edge_index[1], np.arange(N, dtype=np.int32)])
    order = np.argsort(dst, kind="stable")
    src, dst = src[order], dst[order]
    deg = np.bincount(dst, minlength=N)
    starts = np.zeros(N + 1, np.int64)
    np.cumsum(deg, out=starts[1:])

    # degree-balanced node->core assignment (round-robin by degree rank)
    nodes_by_deg = np.lexsort((np.arange(N), deg))
    node_core = np.empty(N, np.int64)
    node_core[nodes_by_deg] = np.arange(N) % NCORES
    dmax = {}
    own_by_core = []
    for c in range(NCORES):
        own = nodes_by_deg[node_core[nodes_by_deg] == c]
        own_by_core.append(own)
        dd, cc = np.unique(deg[own], return_counts=True)
        for d, k in zip(dd, cc):
            dmax[int(d)] = max(dmax.get(int(d), 0), int(k))
    subtiles, NODES_PAD, NSUB = _schedule_v2(dmax)
    NG = NODES_PAD // 128

    proc = np.full((NCORES, NODES_PAD), -1, np.int64)  # slot -> global node
    for c in range(NCORES):
        by_d = {}
        for v in own_by_core[c]:
            by_d.setdefault(int(deg[v]), []).append(int(v))
        for d, take, nstart in subtiles:
            lst = by_d.get(d, [])
            t = min(take, len(lst))
            for j in range(t):
                proc[c, nstart + j] = lst[j]
            by_d[d] = lst[t:]
        assert all(len(v) == 0 for v in by_d.values())
    store = np.full(N, -1, np.int64)
    for c in range(NCORES):
        sel = proc[c] >= 0
        store[proc[c][sel]] = c * NODES_PAD + np.nonzero(sel)[0]
    assert (store >= 0).all()

    ZROW1 = cfg.NT_PAD - 1          # reserved zero row in xl1_full
    esrc1 = np.full((NCORES, 128, NSUB), ZROW1, np.int32)
    esrc2 = np.full((NCORES, 128, NSUB), NCORES * NODES_PAD, np.int32)
    S_all = np.zeros((128, NODES_PAD), NPBF)
    for s, (d, take, nstart) in enumerate(subtiles):
        for j in range(take):
            S_all[j * d:(j + 1) * d, nstart + j] = 1.0
    for c in range(NCORES):
        for s, (d, take, nstart) in enumerate(subtiles):
            for j in range(take):
                v = proc[c, nstart + j]
                if v < 0:
                    continue
                e0 = starts[v]
                dv = int(deg[v])
                assert dv == d
                p0 = j * d
                esrc1[c, p0:p0 + dv, s] = src[e0:e0 + dv]
                esrc2[c, p0:p0 + dv, s] = store[src[e0:e0 + dv]]

    xT = np.zeros((cfg.IN, cfg.NT_PAD), NPBF)
    xT[:, :N] = x.T.astype(NPBF)
    A1 = np.zeros((cfg.F1, HEADS), np.float32)
    for h in range(HEADS):
        A1[h * cfg.HID:(h + 1) * cfg.HID, h] = W["att1"][h]
    A2 = np.zeros((cfg.F2, HEADS), np.float32)
    for h in range(HEADS):
        A2[h * cfg.OUT:(h + 1) * cfg.OUT, h] = W["att2"][h]
    mean1m = np.zeros((128, cfg.HID), np.float32)
    mean1m[np.arange(128), np.arange(128) % cfg.HID] = 0.125
    mean2m = np.zeros((128, cfg.OUT), np.float32)
    mean2m[np.arange(128), np.arange(128) % cfg.OUT] = 0.125
    e1m = np.zeros((8, cfg.F1), np.float32)
    e1m[np.arange(cfg.F1) // cfg.HID, np.arange(cfg.F1)] = 1.0
    e2m = np.zeros((8, cfg.F2), np.float32)
    e2m[np.arange(cfg.F2) // cfg.OUT, np.arange(cfg.F2)] = 1.0

    in_maps = []
    for c in range(NCORES):
        xTo = np.zeros((cfg.IN, NODES_PAD), NPBF)
        sel = proc[c] >= 0
        xTo[:, sel] = x[proc[c][sel]].T.astype(NPBF)
        ndead = NODES_PAD - int(sel.sum())
        v = np.maximum(W["b1"], 0.0).astype(NPBF).astype(np.float32)
        dstat = np.stack([ndead * v, ndead * v * v], 1).astype(np.float32)
        in_maps.append({
            "xT": xT, "xTo": xTo, "dstat": dstat,
            "S_all": S_all,
            "mean1m": mean1m.astype(NPBF), "mean2m": mean2m.astype(NPBF),
            "e1m": e1m.astype(NPBF), "e2m": e2m.astype(NPBF),
            "esrc1": np.ascontiguousarray(esrc1[c]),
            "esrc2": np.ascontiguousarray(esrc2[c]),
            "Wl1": W["Wl1"].astype(NPBF), "Wr1": W["Wr1"].astype(NPBF),
            "Wl2": W["Wl2"].astype(NPBF), "Wr2": W["Wr2"].astype(NPBF),
            "A1": A1.astype(NPBF), "A2": A2.astype(NPBF),
            "gb1": np.stack([W["gamma1"], W["beta1"]], 1).astype(np.float32),
            "gb2": np.stack([W["gamma2"], W["beta2"]], 1).astype(np.float32),
            "b1c": W["b1"].reshape(-1, 1).astype(np.float32),
            "b2c": W["b2"].reshape(-1, 1).astype(np.float32),
        })
    meta = dict(NODES_PAD=NODES_PAD, NSUB=NSUB, NG=NG, subtiles=subtiles,
                proc=proc, in_maps=in_maps)
    return meta


def _build(cfg, meta):
    NODES_PAD, NSUB, NG = meta["NODES_PAD"], meta["NSUB"], meta["NG"]
    subtiles = meta["subtiles"]
    IN, F1, F2, HID, OUT = cfg.IN, cfg.F1, cfg.F2, cfg.HID, cfg.OUT
    NTP = cfg.NT_PAD
    RECIP_N = 1.0 / cfg.N
    K0, K1 = cfg.KT
    kt_sizes = [K0] + ([K1] if K1 else [])

    nc = bacc.Bacc("TRN2", target_bir_lowering=False, debug=False,
                   num_devices=NCORES)
    din = {}
    for name, shape, dt in [
            ("xT", [IN, NTP], BF), ("xTo", [IN, NODES_PAD], BF),
            ("dstat", [HID, 2], F32), ("S_all", [128, NODES_PAD], BF),
            ("esrc1", [128, NSUB], I32), ("esrc2", [128, NSUB], I32),
            ("Wl1", [IN, F1], BF), ("Wr1", [IN, F1], BF),
            ("Wl2", [HID, F2], BF), ("Wr2", [HID, F2], BF),
            ("A1", [F1, HEADS], BF), ("A2", [F2, HEADS], BF),
            ("gb1", [IN, 2], F32), ("gb2", [HID, 2], F32),
            ("b1c", [HID, 1], F32), ("b2c", [OUT, 1], F32),
            ("mean1m", [128, HID], BF), ("mean2m", [128, OUT], BF),
            ("e1m", [8, F1], BF), ("e2m", [8, F2], BF)]:
        din[name] = nc.dram_tensor(name, shape, dt, kind="ExternalInput")
    outT = nc.dram_tensor("outT", [OUT, NODES_PAD], F32, kind="ExternalOutput")

    xl1_full = nc.dram_tensor("xl1_full", [NTP, F1], BF)
    xl2_full = nc.dram_tensor("xl2_full", [NCORES * NODES_PAD + 128, F2], BF)
    st1_in = nc.dram_tensor("st1_in", [IN, 2], F32)
    st1_out = nc.dram_tensor("st1_out", [IN, 2], F32)
    ag_in = nc.dram_tensor("ag_in", [HID + 2, NODES_PAD], BF)
    ag_out = nc.dram_tensor("ag_out", [NCORES * (HID + 2), NODES_PAD], BF,
                            addr_space="Shared")

    import contextlib
    with tile.TileContext(nc) as tc:
        ctx = contextlib.ExitStack()
        with ctx:
            cpool = ctx.enter_context(tc.tile_pool(name="const", bufs=1))
            rpool = ctx.enter_context(tc.tile_pool(name="resident", bufs=1))

            # ---------- constants / resident inputs ----------
            ident = cpool.tile([128, 128], BF)
            make_identity(nc, ident[:])
            epsb = cpool.tile([128, 1], F32, tag="epsb")
            nc.vector.memset(epsb[:], BN_EPS)
            msh1 = cpool.tile([128, 1], F32, tag="msh1")
            nc.vector.memset(msh1[:], -float(cfg.M1))
            msh2 = cpool.tile([128, 1], F32, tag="msh2")
            nc.vector.memset(msh2[:], -float(cfg.M2))
            nslope = cpool.tile([128, 1], F32, tag="nslope")
            nc.vector.memset(nslope[:], -(1.0 - NEG_SLOPE))

            def load_res(name, shape, dt=BF):
                t = rpool.tile(shape, dt, tag=name, name=name)
                nc.sync.dma_start(out=t[:], in_=din[name].ap())
                return t

            S_sb = load_res("S_all", [128, NODES_PAD])
            esrc1_sb = load_res("esrc1", [128, NSUB], I32)
            esrc2_sb = load_res("esrc2", [128, NSUB], I32)
            a1h, a2h = [], []
            for h in range(F1 // 128):
                t = rpool.tile([128, HEADS], BF, tag=f"A1_{h}",
                               name=f"A1_{h}")
                nc.sync.dma_start(
                    out=t[:], in_=din["A1"].ap()[h * 128:(h + 1) * 128, :])
                a1h.append(t)
            for h in range(F2 // 128):
                t = rpool.tile([128, HEADS], BF, tag=f"A2_{h}",
                               name=f"A2_{h}")
                nc.sync.dma_start(
                    out=t[:], in_=din["A2"].ap()[h * 128:(h + 1) * 128, :])
                a2h.append(t)
            mean1 = load_res("mean1m", [128, HID])
            mean2 = load_res("mean2m", [128, OUT])
            e1full = load_res("e1m", [8, F1])
            e2full = load_res("e2m", [8, F2])
            e1h = [e1full[:, h * 128:(h + 1) * 128] for h in range(F1 // 128)]
            e2h = [e2full[:, h * 128:(h + 1) * 128] for h in range(F2 // 128)]
            b1c_sb = load_res("b1c", [HID, 1], F32)
            b2c_sb = load_res("b2c", [OUT, 1], F32)
            dstat_sb = load_res("dstat", [HID, 2], F32)
            wl1_sb, wr1_sb = [], []
            for ki, ks in enumerate(kt_sizes):
                for wname, lst in (("Wl1", wl1_sb), ("Wr1", wr1_sb)):
                    t = rpool.tile([ks, F1], BF, tag=f"{wname}_{ki}",
                                   name=f"{wname}_{ki}")
                    nc.sync.dma_start(
                        out=t[:],
                        in_=din[wname].ap()[ki * 128:ki * 128 + ks, :])
                    lst.append(t)
            wl2_sb = load_res("Wl2", [HID, F2])
            wr2_sb = load_res("Wr2", [HID, F2])

            # ---------- phase A: BN1 stats + AllReduce ----------
            pXT_cm = tc.tile_pool(name="pXT", bufs=1)
            pXT = pXT_cm.__enter__()
            xTo_sb = []
            for ki, ks in enumerate(kt_sizes):
                t = pXT.tile([ks, NODES_PAD], BF, tag=f"xTo{ki}",
                             name=f"xTo{ki}")
                nc.sync.dma_start(
                    out=t[:], in_=din["xTo"].ap()[ki * 128:ki * 128 + ks, :])
                xTo_sb.append(t)
            with tc.tile_pool(name="pA", bufs=2) as pa, \
                 tc.tile_pool(name="pAs", bufs=1) as pas:
                for ki, ks in enumerate(kt_sizes):
                    st = pa.tile([ks, 2], F32, tag="st")
                    nc.vector.tensor_reduce(out=st[:, 0:1], in_=xTo_sb[ki][:],
                                            axis=mybir.AxisListType.X,
                                            op=mybir.AluOpType.add)
                    scr = pas.tile([ks, NODES_PAD], BF, tag="scr")
                    nc.scalar.activation(
                        out=scr[:], in_=xTo_sb[ki][:],
                        func=mybir.ActivationFunctionType.Square,
                        accum_out=st[:, 1:2])
                    nc.sync.dma_start(
                        out=st1_in.ap()[ki * 128:ki * 128 + ks, :], in_=st[:])
            nc.gpsimd.collective_compute(
                "AllReduce", mybir.AluOpType.add,
                ins=[st1_in.ap()], outs=[st1_out.ap()],
                replica_groups=[list(range(NCORES))])

            # fold stats -> s1, t1 per K tile (applied as Act scale/bias)
            s1_t, t1_t = [], []
            with tc.tile_pool(name="pB", bufs=1) as pb:
                for ki, ks in enumerate(kt_sizes):
                    stg = pb.tile([ks, 2], F32, tag=f"stg{ki}", name=f"stg{ki}")
                    nc.sync.dma_start(
                        out=stg[:],
                        in_=st1_out.ap()[ki * 128:ki * 128 + ks, :])
                    gb = pb.tile([ks, 2], F32, tag=f"gb{ki}", name=f"gb{ki}")
                    nc.sync.dma_start(
                        out=gb[:],
                        in_=din["gb1"].ap()[ki * 128:ki * 128 + ks, :])
                    mean = pb.tile([ks, 1], F32, tag=f"mean{ki}",
                                   name=f"mean{ki}")
                    nc.vector.tensor_scalar(out=mean[:], in0=stg[:, 0:1],
                                            scalar1=RECIP_N, scalar2=None,
                                            op0=mybir.AluOpType.mult)
                    q = pb.tile([ks, 1], F32, tag=f"q{ki}", name=f"q{ki}")
                    nc.vector.tensor_scalar(out=q[:], in0=stg[:, 1:2],
                                            scalar1=RECIP_N, scalar2=None,
                                            op0=mybir.AluOpType.mult)
                    m2 = pb.tile([ks, 1], F32, tag=f"m2{ki}", name=f"m2{ki}")
                    nc.vector.tensor_tensor(out=m2[:], in0=mean[:],
                                            in1=mean[:],
                                            op=mybir.AluOpType.mult)
                    var = pb.tile([ks, 1], F32, tag=f"var{ki}",
                                  name=f"var{ki}")
                    nc.vector.tensor_tensor(out=var[:], in0=q[:], in1=m2[:],
                                            op=mybir.AluOpType.subtract)
                    sd = pb.tile([ks, 1], F32, tag=f"sd{ki}", name=f"sd{ki}")
                    nc.scalar.activation(
                        out=sd[:], in_=var[:],
                        func=mybir.ActivationFunctionType.Sqrt,
                        bias=epsb[:ks, :1])
                    rstd = pb.tile([ks, 1], F32, tag=f"rstd{ki}",
                                   name=f"rstd{ki}")
                    nc.vector.reciprocal(rstd[:], sd[:])
                    s1 = rpool.tile([ks, 1], F32, tag=f"s1_{ki}",
                                    name=f"s1_{ki}")
                    nc.vector.tensor_tensor(out=s1[:], in0=gb[:, 0:1],
                                            in1=rstd[:],
                                            op=mybir.AluOpType.mult)
                    ms = pb.tile([ks, 1], F32, tag=f"ms{ki}", name=f"ms{ki}")
                    nc.vector.tensor_tensor(out=ms[:], in0=mean[:], in1=s1[:],
                                            op=mybir.AluOpType.mult)
                    t1 = rpool.tile([ks, 1], F32, tag=f"t1_{ki}",
                                    name=f"t1_{ki}")
                    nc.vector.tensor_tensor(out=t1[:], in0=gb[:, 1:2],
                                            in1=ms[:],
                                            op=mybir.AluOpType.subtract)
                    s1_t.append(s1)
                    t1_t.append(t1)

            # ---------- phase A2: xn_own + xr1T resident ----------
            xn_own = []
            for ki, ks in enumerate(kt_sizes):
                t = pXT.tile([ks, NODES_PAD], BF, tag=f"xn{ki}",
                             name=f"xn{ki}")
                nc.scalar.activation(
                    out=t[:], in_=xTo_sb[ki][:],
                    func=mybir.ActivationFunctionType.Identity,
                    scale=s1_t[ki][:, :1], bias=t1_t[ki][:, :1])
                xn_own.append(t)
            nh1 = F1 // 128
            XRPAD = NODES_PAD + 128
            xr1T = [rpool.tile([128, XRPAD], BF, tag=f"xr1T_{h}",
                               name=f"xr1T_{h}") for h in range(nh1)]
            GB = 4   # groups per psum bank
            with tc.tile_pool(name="pC", bufs=2, space="PSUM") as pc:
                for h in range(nh1):
                    nc.vector.memset(xr1T[h][:, NODES_PAD:], 0.0)
                for g0 in range(0, NG, GB):
                    gn = min(GB, NG - g0)
                    for h in range(nh1):
                        ps = pc.tile([128, GB * 128], F32, space="PSUM",
                                     tag="xr1p")
                        for gi in range(gn):
                            cols = slice((g0 + gi) * 128, (g0 + gi + 1) * 128)
                            for ki, ks in enumerate(kt_sizes):
                                nc.tensor.matmul(
                                    out=ps[:, gi * 128:(gi + 1) * 128],
                                    lhsT=wr1_sb[ki][:, h * 128:(h + 1) * 128],
                                    rhs=xn_own[ki][:, cols],
                                    start=(ki == 0),
                                    stop=(ki == len(kt_sizes) - 1))
                        dstc = xr1T[h][:, g0 * 128:(g0 + gn) * 128]
                        if h % 2 == 0:
                            nc.vector.tensor_copy(out=dstc,
                                                  in_=ps[:, :gn * 128])
                        else:
                            nc.scalar.copy(out=dstc, in_=ps[:, :gn * 128])

            pXT_cm.__exit__(None, None, None)

            # ---------- phase A3: xl1_full table (batched) ----------
            TB = 8   # node tiles per batch
            n_t1 = NTP // 128
            with tc.tile_pool(name="pD", bufs=3) as pd, \
                 tc.tile_pool(name="pDp", bufs=2, space="PSUM") as pdp:
                for t0 in range(0, n_t1, TB):
                    tn = min(TB, n_t1 - t0)
                    cols = slice(t0 * 128, (t0 + tn) * 128)
                    lhs = []
                    for ki, ks in enumerate(kt_sizes):
                        lt = pd.tile([ks, TB * 128], BF, tag=f"xl1l{ki}",
                                     name=f"xl1l{ki}")
                        nc.sync.dma_start(
                            out=lt[:, :tn * 128],
                            in_=din["xT"].ap()[ki * 128:ki * 128 + ks, cols])
                        ln = pd.tile([ks, TB * 128], BF, tag=f"xl1n{ki}",
                                     name=f"xl1n{ki}")
                        nc.gpsimd.tensor_scalar(
                            out=ln[:, :tn * 128], in0=lt[:, :tn * 128],
                            scalar1=s1_t[ki][:, :1], scalar2=t1_t[ki][:, :1],
                            op0=mybir.AluOpType.mult,
                            op1=mybir.AluOpType.add)
                        lhs.append(ln)
                    ob = pd.tile([128, TB * F1], BF, tag="xl1o", name="xl1o")
                    ps_cur = None
                    for ti in range(tn):
                        half = ti % 2
                        if half == 0:
                            ps_cur = pdp.tile([128, 2 * F1], F32,
                                              space="PSUM", tag="xl1p",
                                              name="xl1p")
                        pslice = ps_cur[:, half * F1:(half + 1) * F1]
                        for ki, ks in enumerate(kt_sizes):
                            nc.tensor.matmul(
                                out=pslice,
                                lhsT=lhs[ki][:, ti * 128:(ti + 1) * 128],
                                rhs=wl1_sb[ki][:],
                                start=(ki == 0),
                                stop=(ki == len(kt_sizes) - 1))
                        if half == 1 or ti == tn - 1:
                            nf = (half + 1) * F1
                            o0 = (ti - half) * F1
                            if ti % 4 < 2:
                                nc.vector.tensor_copy(
                                    out=ob[:, o0:o0 + nf],
                                    in_=ps_cur[:, :nf])
                            else:
                                nc.scalar.copy(
                                    out=ob[:, o0:o0 + nf],
                                    in_=ps_cur[:, :nf])
                    nc.scalar.dma_start(
                        out=xl1_full.ap()[t0 * 128:(t0 + tn) * 128, :]
                            .rearrange("(t p) f -> p t f", p=128),
                        in_=ob[:, :tn * F1].rearrange("p (t f) -> p t f",
                                                      f=F1))

            with tc.tile_pool(name="pZ1", bufs=1) as pz1:
                zrow = pz1.tile([1, F1], BF, tag="zrow")
                nc.vector.memset(zrow[:], 0.0)
                nc.sync.dma_start(out=xl1_full.ap()[NTP - 1:NTP, :],
                                  in_=zrow[:])

            # ---------- edge phase helper ----------
            GATHK = 1   # HW indirect DMA: one offset per partition row

            def edge_phase(F, xfull, esrc_sb, ah, eh, meanm, shift_ap,
                           bias_col, out_cb, xr_list, layer):
                nhalf = F // 128
                groups = {}
                for s, (d, take, nstart) in enumerate(subtiles):
                    groups.setdefault(nstart // 128, []).append(
                        (s, d, take, nstart))
                ggp_bufs = 12 if GATHK <= 2 else (6 if GATHK <= 6 else 3)
                with tc.tile_pool(name=f"gg{layer}", bufs=ggp_bufs) as ggp, \
                     tc.tile_pool(name=f"gz{layer}", bufs=2,
                                  space="PSUM") as gz, \
                     tc.tile_pool(name=f"gl{layer}", bufs=2,
                                  space="PSUM") as gl, \
                     tc.tile_pool(name=f"gn{layer}", bufs=1,
                                  space="PSUM") as gn, \
                     tc.tile_pool(name=f"gs{layer}", bufs=4) as gs:
                    xls_t = {}
                    for s0 in range(0, NSUB, GATHK):
                        sn = min(GATHK, NSUB - s0)
                        xg = ggp.tile([128, GATHK * F], BF, tag="xg",
                                      name="xg")
                        nc.gpsimd.indirect_dma_start(
                            out=xg[:, :sn * F], out_offset=None,
                            in_=xfull.ap(),
                            in_offset=bass.IndirectOffsetOnAxis(
                                ap=esrc_sb[:, s0:s0 + sn], axis=0))
                        for j in range(sn):
                            xls_t[s0 + j] = (xg, j * F)
                    for g in range(NG):
                        subs = groups[g]
                        numT = gn.tile([128, nhalf * 128], F32, space="PSUM",
                                       tag="numT", name="numT")
                        denT = gn.tile([8, 128], F32, space="PSUM",
                                       tag="denT", name="denT")
                        cov = max(st[3] % 128 + st[2] for st in subs)
                        for b0 in range(0, len(subs), 4):
                            batch = subs[b0:b0 + 4]
                            nb = len(batch)
                            zts = []
                            for h in range(nhalf):
                                zt = gz.tile([128, 4 * 128], F32,
                                             space="PSUM", tag=f"zt{h}",
                                             name=f"zt{h}")
                                zts.append(zt)
                            lg = gl.tile([128, 4 * 8], F32, space="PSUM",
                                         tag="lg", name="lg")
                            for si, (s, d, take, nstart) in enumerate(batch):
                                xg, xo = xls_t[s]
                                ncov = take * d
                                for h in range(nhalf):
                                    zsl = zts[h][:, si * 128:(si + 1) * 128]
                                    xgh = xg[:, xo + h * 128:
                                             xo + (h + 1) * 128]
                                    xr_t = xr_list[h]
                                    rep = xr_t[:, nstart:nstart + take, None] \
                                        .broadcast_to([128, take, d])
                                    if ncov == 128:
                                        nc.tensor.matmul(
                                            out=zsl, lhsT=xgh, rhs=ident[:],
                                            start=True, stop=False)
                                        nc.tensor.matmul(
                                            out=zsl, lhsT=ident[:], rhs=rep,
                                            start=False, stop=True)
                                    else:
                                        zla = zts[h][:, si * 128:
                                                     si * 128 + ncov]
                                        zlb = zts[h][:, si * 128 + ncov:
                                                     (si + 1) * 128]
                                        nc.tensor.matmul(
                                            out=zla, lhsT=xgh,
                                            rhs=ident[:, 0:ncov],
                                            start=True, stop=False)
                                        nc.tensor.matmul(
                                            out=zla, lhsT=ident[:], rhs=rep,
                                            start=False, stop=True)
                                        rep2 = xr_t[:, nstart + take:
                                                    nstart + take + 1, None] \
                                            .broadcast_to([128, 1, 128 - ncov])
                                        nc.tensor.matmul(
                                            out=zlb, lhsT=xgh,
                                            rhs=ident[:, ncov:128],
                                            start=True, stop=False)
                                        nc.tensor.matmul(
                                            out=zlb, lhsT=ident[:], rhs=rep2,
                                            start=False, stop=True)
                            es = []
                            for h in range(nhalf):
                                r8 = gs.tile([128, 4 * 128], BF,
                                             tag=f"r8{h}", name=f"r8{h}")
                                nc.scalar.activation(
                                    out=r8[:, :nb * 128],
                                    in_=zts[h][:, :nb * 128],
                                    func=mybir.ActivationFunctionType.Relu,
                                    scale=nslope[:, :1])
                                e_sb = gs.tile([128, 4 * 128], BF,
                                               tag=f"es{h}", name=f"es{h}")
                                nc.vector.tensor_tensor(
                                    out=e_sb[:, :nb * 128],
                                    in0=zts[h][:, :nb * 128],
                                    in1=r8[:, :nb * 128],
                                    op=mybir.AluOpType.add)
                                es.append(e_sb)
                            for si in range(nb):
                                for h in range(nhalf):
                                    nc.tensor.matmul(
                                        out=lg[:, si * 8:(si + 1) * 8],
                                        lhsT=es[h][:, si * 128:(si + 1) * 128],
                                        rhs=ah[h][:, :],
                                        start=(h == 0), stop=(h == nhalf - 1))
                            w4 = gs.tile([128, 4 * 8], BF, tag="w4",
                                         name="w4")
                            nc.scalar.activation(
                                out=w4[:, :nb * 8], in_=lg[:, :nb * 8],
                                func=mybir.ActivationFunctionType.Exp,
                                bias=shift_ap[:, :1])
                            y4 = gs.tile([128, 4 * F], BF, tag="y4",
                                         name="y4")
                            for si, (s, d, take, nstart) in enumerate(batch):
                                xg, xo = xls_t[s]
                                wv = w4[:, si * 8:(si + 1) * 8, None] \
                                    .broadcast_to([128, 8, F // 8])
                                nc.vector.tensor_tensor(
                                    out=y4[:, si * F:(si + 1) * F].rearrange(
                                        "p (a b) -> p a b", a=8),
                                    in0=xg[:, xo:xo + F].rearrange(
                                        "p (a b) -> p a b", a=8),
                                    in1=wv, op=mybir.AluOpType.mult)
                            for si, (s, d, take, nstart) in enumerate(batch):
                                noff = nstart % 128
                                for h in range(nhalf):
                                    nc.tensor.matmul(
                                        out=numT[:, h * 128 + noff:
                                                 h * 128 + noff + take],
                                        lhsT=y4[:, si * F + h * 128:
                                                si * F + (h + 1) * 128],
                                        rhs=S_sb[:, nstart:nstart + take],
                                        start=True, stop=True)
                                nc.tensor.matmul(
                                    out=denT[0:8, noff:noff + take],
                                    lhsT=w4[:, si * 8:(si + 1) * 8],
                                    rhs=S_sb[:, nstart:nstart + take],
                                    start=True, stop=True)
                        if cov < 128:
                            zc = slice(g * 128 + cov, (g + 1) * 128)
                            for h in range(nhalf):
                                nc.tensor.matmul(
                                    out=numT[:, h * 128 + cov:h * 128 + 128],
                                    lhsT=ident[:], rhs=S_sb[:, zc],
                                    start=True, stop=True)
                            nc.tensor.matmul(
                                out=denT[0:8, cov:128],
                                lhsT=ident[:, 0:8], rhs=S_sb[:, zc],
                                start=True, stop=True)
                        # ---- group epilogue ----
                        den_s = gs.tile([8, 128], F32, tag="den_s",
                                        name="den_s")
                        nc.vector.tensor_scalar(
                            out=den_s[:], in0=denT[:], scalar1=1e-30,
                            scalar2=None, op0=mybir.AluOpType.max)
                        drec = gs.tile([8, 128], F32, tag="drec", name="drec")
                        nc.vector.reciprocal(drec[:], den_s[:])
                        drecb = gs.tile([8, 128], BF, tag="drecb",
                                        name="drecb")
                        nc.scalar.copy(out=drecb[:], in_=drec[:])
                        onts = []
                        for h in range(nhalf):
                            rexp = gz.tile([128, 4 * 128], F32, space="PSUM",
                                           tag=f"zt{h}", name=f"rexp{h}")
                            nc.tensor.matmul(out=rexp[:, :128], lhsT=eh[h],
                                             rhs=drecb[:], start=True,
                                             stop=True)
                            rexpb = gs.tile([128, 128], BF, tag=f"rexpb{h}",
                                            name=f"rexpb{h}")
                            nc.scalar.copy(out=rexpb[:], in_=rexp[:, :128])
                            ont = gs.tile([128, 128], BF, tag=f"ont{h}",
                                          name=f"ont{h}")
                            nc.vector.tensor_tensor(
                                out=ont[:],
                                in0=numT[:, h * 128:(h + 1) * 128],
                                in1=rexpb[:], op=mybir.AluOpType.mult)
                            onts.append(ont)
                        cdim = HID if layer == 1 else OUT
                        ot = gl.tile([cdim, 128], F32, space="PSUM",
                                     tag="lg", name="otp")
                        for h in range(nhalf):
                            nc.tensor.matmul(out=ot[:], lhsT=meanm[:, :cdim],
                                             rhs=onts[h][:], start=(h == 0),
                                             stop=(h == nhalf - 1))
                        out_cb(g, ot, bias_col)

            # ---------- phase B: layer-1 edges -> h1T ----------
            h1T = rpool.tile([HID, NODES_PAD], BF)
            oB = ctx.enter_context(tc.tile_pool(name="oB", bufs=2))

            def l1_out(g, ot_psum, bias_col):
                nc.scalar.activation(out=h1T[:, g * 128:(g + 1) * 128],
                                     in_=ot_psum[:],
                                     func=mybir.ActivationFunctionType.Relu,
                                     bias=bias_col[:, :1])

            edge_phase(F1, xl1_full, esrc1_sb, a1h, e1h, mean1, msh1,
                       b1c_sb, l1_out, xr1T, layer=1)

            # ---------- phase C: AllGather h1T + BN2 + xl2 + xr2T ----------
            with tc.tile_pool(name="pE", bufs=2) as pe:
                st2 = pe.tile([HID, 2], F32, tag="st2")
                nc.vector.tensor_reduce(out=st2[:, 0:1], in_=h1T[:],
                                        axis=mybir.AxisListType.X,
                                        op=mybir.AluOpType.add)
                scr2 = pe.tile([HID, NODES_PAD], BF, tag="scr2")
                nc.scalar.activation(
                    out=scr2[:], in_=h1T[:],
                    func=mybir.ActivationFunctionType.Square,
                    accum_out=st2[:, 1:2])
                nc.vector.tensor_tensor(out=st2[:], in0=st2[:],
                                        in1=dstat_sb[:],
                                        op=mybir.AluOpType.subtract)
                nc.sync.dma_start(out=ag_in.ap()[0:HID, :], in_=h1T[:])
                nc.sync.dma_start(out=ag_in.ap()[HID:HID + 1, 0:2 * HID],
                                  in_=st2[:, 0:1].bitcast(BF))
                nc.sync.dma_start(out=ag_in.ap()[HID + 1:HID + 2, 0:2 * HID],
                                  in_=st2[:, 1:2].bitcast(BF))
            nc.gpsimd.collective_compute(
                "AllGather", mybir.AluOpType.bypass,
                ins=[ag_in.ap()], outs=[ag_out.ap()],
                replica_groups=[list(range(NCORES))])

            with tc.tile_pool(name="pF", bufs=1) as pf:
                s2sum = pf.tile([HID, NCORES], F32, tag="s2sum")
                s2sq = pf.tile([HID, NCORES], F32, tag="s2sq")
                agf = ag_out.ap().bitcast(F32)
                for c in range(NCORES):
                    r = c * (HID + 2) + HID
                    nc.sync.dma_start(out=s2sum[:, c:c + 1],
                                      in_=agf[r:r + 1, 0:HID])
                    nc.sync.dma_start(out=s2sq[:, c:c + 1],
                                      in_=agf[r + 1:r + 2, 0:HID])
                stg = pf.tile([HID, 2], F32, tag="stg2")
                nc.vector.tensor_reduce(out=stg[:, 0:1], in_=s2sum[:],
                                        axis=mybir.AxisListType.X,
                                        op=mybir.AluOpType.add)
                nc.vector.tensor_reduce(out=stg[:, 1:2], in_=s2sq[:],
                                        axis=mybir.AxisListType.X,
                                        op=mybir.AluOpType.add)
                gb = pf.tile([HID, 2], F32, tag="gb2")
                nc.sync.dma_start(out=gb[:], in_=din["gb2"].ap())
                mean = pf.tile([HID, 1], F32, tag="mean2")
                nc.vector.tensor_scalar(out=mean[:], in0=stg[:, 0:1],
                                        scalar1=RECIP_N, scalar2=None,
                                        op0=mybir.AluOpType.mult)
                q = pf.tile([HID, 1], F32, tag="q2")
                nc.vector.tensor_scalar(out=q[:], in0=stg[:, 1:2],
                                        scalar1=RECIP_N, scalar2=None,
                                        op0=mybir.AluOpType.mult)
                m2 = pf.tile([HID, 1], F32, tag="m22")
                nc.vector.tensor_tensor(out=m2[:], in0=mean[:], in1=mean[:],
                                        op=mybir.AluOpType.mult)
                var = pf.tile([HID, 1], F32, tag="var2")
                nc.vector.tensor_tensor(out=var[:], in0=q[:], in1=m2[:],
                                        op=mybir.AluOpType.subtract)
                sd = pf.tile([HID, 1], F32, tag="sd2")
                nc.scalar.activation(out=sd[:], in_=var[:],
                                     func=mybir.ActivationFunctionType.Sqrt,
                                     bias=epsb[:HID, :1])
                rstd = pf.tile([HID, 1], F32, tag="rstd2")
                nc.vector.reciprocal(rstd[:], sd[:])
                s2c = rpool.tile([HID, 1], F32, tag="s2c", name="s2c")
                nc.vector.tensor_tensor(out=s2c[:], in0=gb[:, 0:1],
                                        in1=rstd[:],
                                        op=mybir.AluOpType.mult)
                ms = pf.tile([HID, 1], F32, tag="ms2")
                nc.vector.tensor_tensor(out=ms[:], in0=mean[:], in1=s2c[:],
                                        op=mybir.AluOpType.mult)
                t2c = rpool.tile([HID, 1], F32, tag="t2c", name="t2c")
                nc.vector.tensor_tensor(out=t2c[:], in0=gb[:, 1:2], in1=ms[:],
                                        op=mybir.AluOpType.subtract)

            # xl2_full table (batched, bn via Act on gathered h)
            TB2 = 8
            n_t2 = NODES_PAD // 128
            with tc.tile_pool(name="pG", bufs=3) as pg, \
                 tc.tile_pool(name="pGp", bufs=2, space="PSUM") as pgp:
                for c_src in range(NCORES):
                    r0 = c_src * (HID + 2)
                    for t0 in range(0, n_t2, TB2):
                        tn = min(TB2, n_t2 - t0)
                        cols = slice(t0 * 128, (t0 + tn) * 128)
                        lt = pg.tile([HID, TB2 * 128], BF, tag="xl2l",
                                     name="xl2l")
                        nc.sync.dma_start(out=lt[:, :tn * 128],
                                          in_=ag_out.ap()[r0:r0 + HID, cols])
                        ln = pg.tile([HID, TB2 * 128], BF, tag="xl2n",
                                     name="xl2n")
                        nc.gpsimd.tensor_scalar(
                            out=ln[:, :tn * 128], in0=lt[:, :tn * 128],
                            scalar1=s2c[:, :1], scalar2=t2c[:, :1],
                            op0=mybir.AluOpType.mult,
                            op1=mybir.AluOpType.add)
                        ob = pg.tile([128, TB2 * F2], BF, tag="xl2o",
                                     name="xl2o")
                        for ti in range(0, tn, 4):
                            t4 = min(4, tn - ti)
                            ps = pgp.tile([128, 4 * F2], F32, space="PSUM",
                                          tag="xl2p", name="xl2p")
                            for tj in range(t4):
                                nc.tensor.matmul(
                                    out=ps[:, tj * F2:(tj + 1) * F2],
                                    lhsT=ln[:, (ti + tj) * 128:
                                            (ti + tj + 1) * 128],
                                    rhs=wl2_sb[:], start=True, stop=True)
                            o0 = ti * F2
                            if (ti // 4) % 2 == 0:
                                nc.vector.tensor_copy(
                                    out=ob[:, o0:o0 + t4 * F2],
                                    in_=ps[:, :t4 * F2])
                            else:
                                nc.scalar.copy(
                                    out=ob[:, o0:o0 + t4 * F2],
                                    in_=ps[:, :t4 * F2])
                        base = c_src * NODES_PAD + t0 * 128
                        nc.scalar.dma_start(
                            out=xl2_full.ap()[base:base + tn * 128, :]
                                .rearrange("(t p) f -> p t f", p=128),
                            in_=ob[:, :tn * F2].rearrange("p (t f) -> p t f",
                                                          f=F2))

            with tc.tile_pool(name="pZ2", bufs=1) as pz2:
                zrow2 = pz2.tile([1, F2], BF, tag="zrow2")
                nc.vector.memset(zrow2[:], 0.0)
                nc.sync.dma_start(
                    out=xl2_full.ap()[NCORES * NODES_PAD:
                                      NCORES * NODES_PAD + 1, :],
                    in_=zrow2[:])
            # h1n (scoped) + xr2T resident
            pHN_cm = tc.tile_pool(name="pHN", bufs=1)
            pHN = pHN_cm.__enter__()
            h1n = pHN.tile([HID, NODES_PAD], BF)
            nc.scalar.activation(out=h1n[:], in_=h1T[:],
                                 func=mybir.ActivationFunctionType.Identity,
                                 scale=s2c[:, :1], bias=t2c[:, :1])
            xr2T_t = rpool.tile([128, XRPAD], BF, tag="xr2T", name="xr2T")
            nc.vector.memset(xr2T_t[:, NODES_PAD:], 0.0)
            with tc.tile_pool(name="pH", bufs=2, space="PSUM") as ph:
                for g0 in range(0, NG, GB):
                    gn = min(GB, NG - g0)
                    ps = ph.tile([128, GB * 128], F32, space="PSUM",
                                 tag="xr2p")
                    for gi in range(gn):
                        cols = slice((g0 + gi) * 128, (g0 + gi + 1) * 128)
                        nc.tensor.matmul(out=ps[:, gi * 128:(gi + 1) * 128],
                                         lhsT=wr2_sb[:], rhs=h1n[:, cols],
                                         start=True, stop=True)
                    if (g0 // GB) % 2 == 0:
                        nc.vector.tensor_copy(
                            out=xr2T_t[:, g0 * 128:(g0 + gn) * 128],
                            in_=ps[:, :gn * 128])
                    else:
                        nc.scalar.copy(
                            out=xr2T_t[:, g0 * 128:(g0 + gn) * 128],
                            in_=ps[:, :gn * 128])

            # ---------- phase D: layer-2 edges -> outT ----------
            pHN_cm.__exit__(None, None, None)
            oD = ctx.enter_context(tc.tile_pool(name="oD", bufs=2))
            OGB = 8
            ost = {}

            def l2_out(g, ot_psum, bias_col):
                if g % OGB == 0:
                    ost["t"] = oD.tile([OUT, OGB * 128], F32, tag="ob",
                                       name="ob")
                    ost["g0"] = g
                j = g - ost["g0"]
                nc.scalar.activation(
                    out=ost["t"][:, j * 128:(j + 1) * 128], in_=ot_psum[:],
                    func=mybir.ActivationFunctionType.Identity,
                    bias=bias_col[:, :1])
                if j == OGB - 1 or g == NG - 1:
                    nc.sync.dma_start(
                        out=outT.ap()[:, ost["g0"] * 128:(g + 1) * 128],
                        in_=ost["t"][:, :(j + 1) * 128])

            edge_phase(F2, xl2_full, esrc2_sb, a2h, e2h, mean2, msh2,
                       b2c_sb, l2_out, [xr2T_t], layer=2)

    nc.compile()
    return nc


_CACHE = {}


def _get_nc(cfg, meta):
    key = (cfg.N, cfg.IN, cfg.HID, cfg.OUT, meta["NSUB"], meta["NODES_PAD"])
    if key not in _CACHE:
        _CACHE[key] = _build(cfg, meta)
    return _CACHE[key]


def run(cfg, inputs):
    x = np.asarray(inputs["x"], np.float32)
    ei = np.asarray(inputs["edge_index"], np.int32)
    W = {k: np.asarray(inputs[k], np.float32) for k in
         ("Wl1", "Wr1", "att1", "b1", "gamma1", "beta1",
          "Wl2", "Wr2", "att2", "b2", "gamma2", "beta2")}
    meta = _preprocess(cfg, x, ei, W)
    nc = _get_nc(cfg, meta)
    res = run_bass_kernel_spmd(nc, meta["in_maps"],
                               core_ids=list(range(NCORES)))
    out = np.empty((cfg.N, cfg.OUT), np.float32)
    proc = meta["proc"]
    for c in range(NCORES):
        oT = res.results[c]["outT"]      # [OUT, NODES_PAD]
        sel = proc[c] >= 0
        out[proc[c][sel]] = oT[:, sel].T
    return out, meta, nc


def kernel(**inputs):
    cfg = Cfg(50000, 200, 32, 16, m1=8.0, m2=10.0)
    out, _, _ = run(cfg, inputs)
    return out
